# revision 47
# baseline (speedup 1.0000x reference)
"""Trainium2 Bass kernel for nn_CoordinateConditioning.

out[i,j,h] = v[i,j]*( (X[i]-X[j])@Wcoord[h] + Wdist[h]*R[i,j] + B*Wmask[h] )
             + C[i,h] + C[j,h]
with X = sum_b coords[b], R[i,j] = sum_b 1/(1+||x_b[i]-x_b[j]||^2),
v = pad/uid mask, C = B*c0 + gathered s_to_c sum.

Key structure exploited: ref_space_uid is sorted, so v[i,j] is a narrow
block-diagonal band.  For each core (256 i-rows) only W (~3) of the 16
j-tiles can contain v!=0 pairs; host computes the per-core window-tile
list from the actual inputs and the device program processes exactly
W "window" tiles (full geometry pipeline) + 16-W "dense" tiles
(out = C_i + C_j only, K=17 matmul with a shared stationary operand).
Per-core j-tile order is a host-chosen permutation sigma_c; the host
inverse-permutes the j-tile blocks when assembling the full output.

DMA layout: TRN2 SDMA engines only spray a transfer across all 16
engines when one side spans ~128 partitions; partition-narrow loads
(17/97 rows) land on ONE engine (~22 GB/s).  So the big constant
patterns are staged to SBUF as [128, f] HWDGE loads (sprayed), then
redistributed SBUF->SBUF with 128-partition sources via gpsimd SWDGE
(also sprayed).  Remaining tiny narrow loads ride the Act HWDGE ring so
their single-engine backlog cannot stall the SP ring that carries the
output stores.

Output is written fp16 (rel tolerance is 2e-2; fp16 rounding ~5e-4),
halving the dominant out-DMA traffic.  PSUM->SBUF copies are split
between ACT and DVE.
"""

import numpy as np
import ml_dtypes
from contextlib import ExitStack

FP8 = ml_dtypes.float8_e4m3fn


def _fp8_hi_lo(v):
    """fp8e4m3 hi/lo split: v ~ hi + lo with |err| <= |v| * 2^-8."""
    v = np.ascontiguousarray(v, dtype=np.float32)
    hi = v.astype(FP8)
    lo = (v - hi.astype(np.float32)).astype(FP8)
    return hi, lo

B, N, T, TOKEN_S, DIM_F, H = 4, 2048, 256, 384, 256, 16
NCORES = 8
IB = N // NCORES          # 256 i rows per core
NJT = N // 128            # 16 j tiles
KB = 17                   # K rows per batch for the r2 matmul
KU = 2                    # uid delta rows
KD = 4                    # rows per D_k
KF = 4 * KB + KU + 3 * KD # 82 total J/I feature rows
BIGM = 4096.0

_CACHE = {}


def _split_hi_lo(v):
    """fp16-exact hi/lo split (hi keeps 10 mantissa bits)."""
    v = np.ascontiguousarray(v, dtype=np.float32)
    hi = (v.view(np.uint32) & np.uint32(0xFFFFE000)).view(np.float32)
    return hi, (v - hi).astype(np.float32)


def _host_tables(inputs):
    I = {k: np.asarray(v) for k, v in inputs.items()}
    x = np.ascontiguousarray(I['atom_coords_noisy'], dtype=np.float32)  # [B,N,3]
    m = I['atom_pad_mask'].reshape(-1).astype(np.float32)               # [N]
    uid = I['ref_space_uid'].reshape(-1).astype(np.float32)             # [N]

    # ---- small linears (replicated) ----
    def ln(v, g, b, eps=1e-5):
        mu = v.mean(-1, keepdims=True)
        var = ((v - mu) ** 2).mean(-1, keepdims=True)
        return (v - mu) / np.sqrt(var + eps) * g + b

    s = np.concatenate([I['s_trunk'], I['s_inputs']], -1).astype(np.float32) @ I['W_single'].T
    fe = np.cos(2 * np.pi * (I['times'][:, None] * I['Wf'][:, 0][None, :] + I['bf'])).astype(np.float32)
    s = s + (ln(fe, I['ln_f_g'], I['ln_f_b']) @ I['Wf2s'].T)[:, None, :]
    s2c = ln(s, I['ln_s_g'], I['ln_s_b']) @ I['Wsc'].T                  # [B,T,1]
    ssum = s2c[:, :, 0].sum(0)                                          # [T]
    tok = I['atom_to_token_idx'].reshape(-1).astype(np.int64)
    S = ssum[tok]                                                       # [N]
    af = np.concatenate([I['ref_pos'][0], I['ref_charge'][0][:, None],
                         I['ref_element'][0]], -1).astype(np.float32)   # [N,132]
    c0 = af @ I['Wa'].T + I['ba']                                       # [N,16]
    C = (B * c0 + S[:, None]).astype(np.float32)                        # [N,16]

    X = x.sum(0)                                                        # [N,3]
    Wc = np.asarray(I['Wcoord'], np.float32)                            # [16,3]
    # device feature maps hold X_j - X_i, the formula needs X_i - X_j -> negate
    wtab = np.stack([-Wc[:, 0], -Wc[:, 1], -Wc[:, 2],
                     np.asarray(I['Wdist'], np.float32)[:, 0],
                     B * np.asarray(I['Wmask'], np.float32)[:, 0]], 0)  # [5,16]

    # ---- J/I feature tables for the per-(j,i) matmuls ----
    n2 = np.einsum('bnk,bnk->bn', x.astype(np.float64), x.astype(np.float64)).astype(np.float32)
    ones = np.ones(N, np.float32)
    jf = np.zeros((KF, N), np.float32)
    itab = np.zeros((KF, N), np.float32)
    for b in range(B):
        r = b * KB
        for k in range(3):
            xh, xl = _split_hi_lo(x[b, :, k])
            jf[r + 4 * k + 0] = xh
            jf[r + 4 * k + 1] = xh
            jf[r + 4 * k + 2] = xl
            jf[r + 4 * k + 3] = xl
            itab[r + 4 * k + 0] = -2.0 * xh
            itab[r + 4 * k + 1] = -2.0 * xl
            itab[r + 4 * k + 2] = -2.0 * xh
            itab[r + 4 * k + 3] = -2.0 * xl
        nh, nl = _split_hi_lo(n2[b])
        jf[r + 12], jf[r + 13] = nh, nl
        itab[r + 12], itab[r + 13] = ones, ones
        jf[r + 14], jf[r + 15] = ones, ones
        itab[r + 14], itab[r + 15] = nh, nl
        jf[r + 16] = ones
        itab[r + 16] = ones
    ru = 4 * KB
    jf[ru] = uid + BIGM * (1.0 - m)
    itab[ru] = ones
    jf[ru + 1] = ones
    itab[ru + 1] = -uid + BIGM * (1.0 - m)
    for k in range(3):
        r = ru + KU + KD * k
        Xh, Xl = _split_hi_lo(X[:, k])
        jf[r + 0], jf[r + 1] = Xh, Xl
        itab[r + 0], itab[r + 1] = ones, ones
        jf[r + 2], jf[r + 3] = ones, ones
        itab[r + 2], itab[r + 3] = -Xh, -Xl

    # ---- constant rhs pattern rows (per 256-col jsub block) ----
    # rows 0..79: geometry block-diag(delta_jp * wtab[t]); 80..95: delta_h
    blk = np.zeros((96, 256), np.float32)
    for t in range(5):
        for jp in range(16):
            blk[t * 16 + jp, jp * 16:(jp + 1) * 16] = wtab[t]
    for hp in range(16):
        blk[80 + hp, hp::16] = 1.0
    pc = np.tile(blk, (1, 8))                                           # [96, 2048]

    # bake the per-tile column permutation p -> j = (p%8)*16 + p//8 into jf
    # so device lhsT slices are plain contiguous (walrus: one free dim only)
    p = np.arange(128)
    perm = (np.arange(N) // 128) * 128 + ((p % 8) * 16 + p // 8)[np.tile(p, N // 128) * 0 + np.arange(N) % 128]
    jf = np.ascontiguousarray(jf[:, perm])

    cflat = C.reshape(1, N * H).astype(np.float32)
    return jf, itab, pc, C, cflat, m.astype(bool), uid


def _windows(m, uid):
    """Per-core window j-tile lists (tiles that can hold v!=0 pairs) and
    the per-core slot->j-tile permutation sigma (window tiles first)."""
    tiles_per_core = []
    for c in range(NCORES):
        sl = slice(c * IB, (c + 1) * IB)
        vi = m[sl]
        if vi.any():
            U = np.unique(uid[sl][vi])
            pj = np.where(m & np.isin(uid, U))[0]
            tiles = sorted(set((pj // 128).tolist()))
        else:
            tiles = []
        tiles_per_core.append(tiles)
    W = max(1, max(len(t) for t in tiles_per_core))
    sigmas = []
    for tiles in tiles_per_core:
        rest = [t for t in range(NJT) if t not in tiles]
        pad = rest[:W - len(tiles)]
        rest2 = rest[W - len(tiles):]
        sigmas.append(np.array(tiles + pad + rest2, np.int64))
    return sigmas, W


def _build_program(W):
    key = ('nc', W)
    if key in _CACHE:
        return _CACHE[key]
    import concourse.bass as bass
    import concourse.bacc as bacc
    import concourse.tile as tile
    from concourse import mybir

    f32 = mybir.dt.float32
    f32r = mybir.dt.float32r
    f16 = mybir.dt.float16
    f8 = mybir.dt.float8e4

    GW = W * 2048            # valid geometry columns in the merged tile

    nc = bacc.Bacc("TRN2", target_bir_lowering=False, debug=False)
    # wide staging payloads (all sprayed as [128, f] loads, then SWDGE-
    # redistributed with 128-partition sources into the narrow operand
    # layouts).  sw1 feeds the dense steps + stage 1, sw2 the window steps.
    # The delta_h pattern is generated on device (17 strided memsets), not
    # loaded: it would otherwise dominate the dense-path critical DMA.
    #   sw1 = [ Cjflat(256) | J4 rows0-15(256) | I4 rows0-15(128) | delta16(16) | pad ]
    #   sw2a = [ geo rows0-63 (W*1024) ]   (8KB rows, the proven-fast shape)
    #   sw2b = [ geo rows64-79 (W*256) | lwc rows0-15(256) | pad ]
    FW1 = 2048
    sw1 = nc.dram_tensor("sw1", [128, FW1], f16, kind="ExternalInput").ap()
    sw2a = nc.dram_tensor("sw2a", [128, W * 1024], f16, kind="ExternalInput").ap()
    FW2B = 2048
    sw2b = nc.dram_tensor("sw2b", [128, FW2B], f16, kind="ExternalInput").ap()
    outp = nc.dram_tensor("outp", [IB, N * H], f16, kind="ExternalOutput").ap()

    with tile.TileContext(nc) as tc:
        with ExitStack() as ctx:
            cpool = ctx.enter_context(tc.tile_pool(name="const", bufs=1))
            J4 = cpool.tile([KB, B * W * 128], f16, tag="J4")
            I4 = cpool.tile([KB, B * IB], f16, tag="I4")
            J3 = cpool.tile([KD, 3 * W * 128], f16, tag="J3")
            I3 = cpool.tile([KD, 3 * IB], f16, tag="I3")
            Ju = cpool.tile([KU, W * 128], f32r, tag="Ju")
            Iu = cpool.tile([KU, IB], f32r, tag="Iu")
            # window rhs pattern tile (fp16): rows 0..16 = [Cj; delta_h],
            # rows 17..96 = geometry; only window columns (< GW) are used
            PB = cpool.tile([97, GW], f16, tag="PB")
            # dense rhs pattern tile (fp8e4, DoubleRow): row 0 = Cj_hi,
            # row 1 = Cj_lo, rows 2..17 = delta_h; the DoubleRow pair dim is
            # stride-0 on this operand
            PB8 = cpool.tile([18, NJT * 2048], f8, tag="PB8")
            # dense lhsT fp8 pairs: row 0/1 = (1,0), rows 2..17 = (CiH, CiL)
            LD8 = cpool.tile([18, 2 * IB], f8, tag="LD8")
            # window lhsT helper: row 0 = ones, rows 1..17 = Ci^T
            Ld = cpool.tile([17, IB], f16, tag="Ld")
            # staging tiles for the wide loads (sprayed across all 16 SDMA
            # engines), redistributed below with 128-partition sources
            SW1 = cpool.tile([128, FW1], f16, tag="SW1")
            SW2A = cpool.tile([128, W * 1024], f16, tag="SW2A")
            SW2B = cpool.tile([128, FW2B], f16, tag="SW2B")
            SD = cpool.tile([128, 1024], f16, tag="SD")
            SD8 = cpool.tile([128, 4096], f8, tag="SD8")

            # wide staged loads on the SP HWDGE ring (shared with stores)
            nc.sync.dma_start(SW1[:, :], sw1[:, :])
            nc.sync.dma_start(SW2A[:, :], sw2a[:, :])
            nc.sync.dma_start(SW2B[:, :], sw2b[:, :])

            # delta_h staging, generated on device: partition p holds the
            # 16-periodic one-hot(p//8) pattern, expanded by DVE broadcast
            # (fp16 copy for the window tile, fp8 cast for the dense tile)
            nc.vector.tensor_copy(
                SD[:, :].rearrange("p (a s) -> p a s", s=16),
                SW1[:, 640:656].unsqueeze(1).broadcast_to([128, 64, 16]))
            nc.vector.tensor_copy(
                SD8[:, :].rearrange("p (a s) -> p a s", s=16),
                SW1[:, 640:656].unsqueeze(1).broadcast_to([128, 256, 16]))

            # Everything is SWDGE-redistributed out of the wide staging
            # tiles; there are no partition-narrow DRAM loads at all (they
            # hot-spot one SDMA engine and gate every completion).  SWDGE
            # assigns each dma_start to ONE SDMA engine (~25 GB/s), so the
            # big patterns move as ~128KB pieces across several
            # instructions.  Order tracks consumption: dense-path operands
            # first, stage-1 operands, then the window-path operands.
            def chunk16(c):
                nc.gpsimd.dma_start(
                    PB[0:1, c * 4096:(c + 1) * 4096].rearrange(
                        "r (q f) -> r q f", q=16),
                    SW1[c * 16:(c + 1) * 16, 0:256])
                nc.gpsimd.dma_start(
                    PB[1:17, c * 4096:(c + 1) * 4096].rearrange(
                        "r (q f) -> r q f", q=8),
                    SD[:, c * 512:(c + 1) * 512])

            def chunk8(c):
                nc.gpsimd.dma_start(
                    PB8[2:18, c * 4096:(c + 1) * 4096].rearrange(
                        "r (q f) -> r q f", q=8),
                    SD8[:, c * 512:(c + 1) * 512])

            nc.gpsimd.dma_start(
                LD8[2:18, :].rearrange("r (q f) -> r q f", q=8),
                SW1[:, 1088:1120].bitcast(f8))
            nc.gpsimd.dma_start(
                LD8[0:2, :].rearrange("r (q f) -> r q f", q=64),
                SW1[:, 1120:1124].bitcast(f8))
            nc.gpsimd.dma_start(
                PB8[0:2, :].rearrange("r (q f) -> r q f", q=64),
                SW1[:, 832:1088].bitcast(f8))
            # first dense steps gate on these; small pieces complete in
            # ~1.5-3us each on their SWDGE engines
            nc.gpsimd.dma_start(
                PB8[2:18, 8192:10240].rearrange("r (q f) -> r q f", q=8),
                SD8[:, 1024:1280])
            nc.gpsimd.dma_start(
                PB8[2:18, 10240:12288].rearrange("r (q f) -> r q f", q=8),
                SD8[:, 1280:1536])
            nc.gpsimd.dma_start(
                PB8[2:18, 12288:16384].rearrange("r (q f) -> r q f", q=8),
                SD8[:, 1536:2048])
            nc.gpsimd.dma_start(
                J4[0:16, :].rearrange("r (q f) -> r q f", q=8),
                SW1[:, 256:512])
            nc.gpsimd.dma_start(
                I4[0:16, :].rearrange("r (q f) -> r q f", q=8),
                SW1[:, 512:640])
            nc.gpsimd.dma_start(
                J4[16:17, :].rearrange("r (q f) -> r q f", q=128),
                SW1[:, 656:672])
            nc.gpsimd.dma_start(
                I4[16:17, :].rearrange("r (q f) -> r q f", q=128),
                SW1[:, 672:680])
            nc.gpsimd.dma_start(
                Ju[:, :].bitcast(f16).rearrange("r (q f) -> r q f", q=64),
                SW1[:, 730:746])
            nc.gpsimd.dma_start(
                Iu[:, :].bitcast(f16).rearrange("r (q f) -> r q f", q=64),
                SW1[:, 746:754])
            nc.gpsimd.dma_start(
                J3[:, :].rearrange("r (q f) -> r q f", q=32),
                SW1[:, 754:802])
            nc.gpsimd.dma_start(
                I3[:, :].rearrange("r (q f) -> r q f", q=32),
                SW1[:, 802:826])
            chunk8(4)
            chunk8(5)
            # window lhsT per slot: rows 0..17 = [ones; Ci], 17..97 geometry (repack)
            Lw = []
            for s in range(W):
                Lt = cpool.tile([97, 2048], f16, tag=f"Lw{s}")
                nc.gpsimd.dma_start(
                    Lt[0:16, :].rearrange("r (q f) -> r q f", q=8),
                    SW2B[:, W * 256:W * 256 + 256])
                nc.gpsimd.dma_start(
                    Lt[16:17, :].rearrange("r (q f) -> r q f", q=128),
                    SW1[:, 714:730])
                Lw.append(Lt)
            chunk8(6)
            chunk8(7)
            nc.gpsimd.dma_start(
                PB[17:81, 0:GW].rearrange("r (q f) -> r q f", q=2),
                SW2A[:, :])
            nc.gpsimd.dma_start(
                PB[81:97, 0:GW].rearrange("r (q f) -> r q f", q=8),
                SW2B[:, 0:W * 256])
            chunk16(0)
            chunk16(1)

            psO = ctx.enter_context(tc.tile_pool(name="psO", bufs=4, space="PSUM"))
            wk = ctx.enter_context(tc.tile_pool(name="wk", bufs=2))
            wkr = ctx.enter_context(tc.tile_pool(name="wkr", bufs=4))
            stg = ctx.enter_context(tc.tile_pool(name="stg", bufs=5))
            dstv = outp.rearrange("(t p) nh -> p t nh", t=2)

            def emit_step(k, slot):
                """One output step: 8 (dense) or 16 (window) matmuls ->
                4 PSUM->SBUF fp16 copies -> one 1MB store."""
                dense = slot >= W
                # while stage-1's chain owns the DVE queue, ACT takes all
                # 4 PSUM->SBUF copies; otherwise split 2/2
                a = 3 if 4 <= k < 9 else 2
                qidx = 0
                st = stg.tile([128, 4096], f16, tag="st", name="st")
                for it in range(2):
                    for g in range(2):
                        po = psO.tile([128, 1024], f32, tag="po", name="po")
                        if dense:
                            for jl in range(2):
                                c0 = slot * 2048 + g * 1024 + jl * 512
                                nc.tensor.matmul(
                                    po[:, jl * 512:(jl + 1) * 512],
                                    LD8[0:18, :].rearrange(
                                        "k (two m) -> k two m",
                                        two=2)[:, :, it * 128:(it + 1) * 128],
                                    PB8[0:18, c0:c0 + 512].unsqueeze(1)
                                        .broadcast_to([18, 2, 512]),
                                    start=True, stop=True,
                                    perf_mode=mybir.MatmulPerfMode.DoubleRow)
                        else:
                            for jl in range(4):
                                js = g * 4 + jl
                                base = js * 256 + it * 128
                                nc.tensor.matmul(
                                    po[:, jl * 256:(jl + 1) * 256],
                                    Lw[slot][0:97, base:base + 128],
                                    PB[0:97, slot * 2048 + js * 256:slot * 2048 + (js + 1) * 256],
                                    start=True, stop=True)
                        dst = st[:, it * 2048 + g * 1024:it * 2048 + (g + 1) * 1024]
                        if qidx < a:
                            nc.scalar.copy(dst, po[:, :])
                        else:
                            nc.vector.tensor_copy(dst, po[:, :])
                        qidx += 1
                    nc.sync.dma_start(
                        dstv[:, it:it + 1, slot * 2048:(slot + 1) * 2048],
                        st[:, it * 2048:(it + 1) * 2048])

            def emit_stage1():
                """Geometry features for the W window slots."""
                for s in range(W):
                    ps1 = psO.tile([128, 1024], f32, tag="po", name="ps1")
                    for b in range(B):
                        nc.tensor.matmul(ps1[:, b * 256:(b + 1) * 256],
                                         J4[:, (b * W + s) * 128:(b * W + s + 1) * 128],
                                         I4[:, b * IB:(b + 1) * IB],
                                         start=True, stop=True)
                    rc = wkr.tile([128, 1024], f32, tag="rc", name="rc")
                    nc.vector.reciprocal_approx_fast(rc[:, :], ps1[:, :])
                    ps2 = psO.tile([128, 1024], f32, tag="po", name="ps2")
                    nc.tensor.matmul(ps2[:, 0:256],
                                     Ju[:, s * 128:(s + 1) * 128], Iu[:, :],
                                     start=True, stop=True)
                    for k in range(3):
                        nc.tensor.matmul(ps2[:, 256 + k * 256:512 + k * 256],
                                         J3[:, (k * W + s) * 128:(k * W + s + 1) * 128],
                                         I3[:, k * IB:(k + 1) * IB],
                                         start=True, stop=True)
                    # drain ps2 to SBUF immediately (ACT) so the PSUM bank
                    # frees without waiting for the whole DVE chain; the PE
                    # can then roll straight into the next output step
                    pv = wkr.tile([128, 1024], f32, tag="pv", name="pv")
                    nc.scalar.copy(pv[:, :], ps2[:, :])
                    r2h = wk.tile([128, 512], f32, tag="r2h", name="r2h")
                    Rt = wk.tile([128, 256], f32, tag="Rt", name="Rt")
                    # gpsimd stays DMA-only (tensor ops there force a 14us
                    # ucode LOAD_LIB swap mid-kernel)
                    nc.vector.tensor_add(r2h[:, :], rc[:, 0:512], rc[:, 512:1024])
                    nc.vector.tensor_add(Rt[:, :], r2h[:, 0:256], r2h[:, 256:512])

                    F5 = wk.tile([128, 1280], f16, tag="F5", name="F5")
                    vt = wk.tile([128, 256], f32, tag="vt", name="vt")
                    nc.vector.tensor_scalar(vt[:, :], pv[:, 0:256], 0.0, None,
                                            op0=mybir.AluOpType.is_equal)
                    for k in range(3):
                        nc.vector.tensor_mul(F5[:, k * 256:(k + 1) * 256],
                                             vt[:, :], pv[:, 256 + k * 256:512 + k * 256])
                    nc.vector.tensor_mul(F5[:, 768:1024], vt[:, :], Rt[:, :])
                    nc.vector.tensor_copy(F5[:, 1024:1280], vt[:, :])

                    # repack: permuted partitions make each feature's src contiguous
                    for t in range(5):
                        dst = Lw[s][17 + t * 16:17 + (t + 1) * 16, :].rearrange(
                            "k (a i) -> k a i", a=8)
                        nc.gpsimd.dma_start(dst, F5[:, t * 256:(t + 1) * 256])

            # ---- emission order: a few dense steps first (their operands
            # arrive earliest), then stage 1, then the rest with the
            # windows interleaved ----
            dense_list = list(range(W, NJT))
            for k, slot in enumerate(dense_list[:4]):
                emit_step(k, slot)
            emit_stage1()
            post = dense_list[4:9]
            for w in range(W):
                post.append(w)
                if 9 + w < len(dense_list):
                    post.append(dense_list[9 + w])
            for k, slot in enumerate(post):
                emit_step(4 + k, slot)
    nc.compile()
    _CACHE[key] = nc
    return nc


def make_in_maps(inputs):
    jf, itab, pc, C, cflat, m, uid = _host_tables(inputs)
    sigmas, W = _windows(m, uid)
    pcw16 = pc.astype(np.float16)             # [96, 2048]
    pcd16 = np.ascontiguousarray(pc[80:96]).astype(np.float16)  # [16, 2048] delta
    in_maps = []
    ru = 4 * KB
    # geometry rows tiled x W slots: [80, W*2048]
    geo = np.tile(pcw16[0:80], (1, W))
    wg1_all = np.ascontiguousarray(geo[0:64]).reshape(64, 2, W * 1024).reshape(128, W * 1024)
    wg2_all = np.ascontiguousarray(geo[64:80]).reshape(16, 8, W * 256).reshape(128, W * 256)

    def wideN(a):
        """[R, F] -> [128, R*F//128] staging layout (row r piece q -> part
        r*(128//R)+q), for R dividing 128."""
        r, f = a.shape
        q = 128 // r
        return np.ascontiguousarray(a).reshape(r, q, f // q).reshape(128, f // q)

    for c in range(NCORES):
        sl = slice(c * IB, (c + 1) * IB)
        sg = sigmas[c]
        jfw = np.concatenate([jf[:, t * 128:(t + 1) * 128] for t in sg[:W]], axis=1)
        cflP = np.concatenate([cflat[:, t * 2048:(t + 1) * 2048] for t in sg], axis=1)
        citc = np.ascontiguousarray(C.T[:, sl]).astype(np.float16)   # [16, IB]
        cf16 = np.ascontiguousarray(cflP).astype(np.float16)
        ldc = np.concatenate([np.ones((1, IB), np.float16), citc], 0)
        lwc = np.concatenate([np.ones((1, 2048), np.float16),
                              np.tile(citc, (1, 8))], 0)
        ifc = itab[:, sl]
        j4 = np.concatenate([jfw[b * KB:(b + 1) * KB] for b in range(B)],
                            axis=1).astype(np.float16)
        i4 = np.concatenate([ifc[b * KB:(b + 1) * KB] for b in range(B)],
                            axis=1).astype(np.float16)
        j3 = np.concatenate([jfw[ru + KU + KD * k:ru + KU + KD * (k + 1)]
                             for k in range(3)], axis=1).astype(np.float16)
        i3 = np.concatenate([ifc[ru + KU + KD * k:ru + KU + KD * (k + 1)]
                             for k in range(3)], axis=1).astype(np.float16)
        ju = np.ascontiguousarray(jfw[ru:ru + KU], np.float32)   # f32r bytes
        iu = np.ascontiguousarray(ifc[ru:ru + KU], np.float32)
        onehot = np.zeros((128, 16), np.float16)
        onehot[np.arange(128), np.arange(128) // 8] = 1.0
        # fp8 dense operands: Cj/Ci hi+lo planes (DoubleRow pairs)
        cjh, cjl = _fp8_hi_lo(cflP)                     # [1, 32768] each
        cjhl = np.concatenate([cjh, cjl], 0)            # [2, 32768] fp8
        cif = np.ascontiguousarray(C.T[:, sl], np.float32)  # [16, IB]
        cih, cil = _fp8_hi_lo(cif)
        ld8 = np.zeros((18, 2, IB), FP8)
        ld8[0, 0] = FP8(1.0)
        ld8[1, 0] = FP8(1.0)
        ld8[2:, 0] = cih
        ld8[2:, 1] = cil
        ld8 = ld8.reshape(18, 2 * IB)
        # one wide staging tensor holds every operand; segment offsets must
        # match the device-side redistributes in _build_program
        sw1 = np.zeros((128, 2048), np.float16)
        sw1[:, 0:256] = cf16.reshape(128, 256)
        sw1[:, 256:512] = wideN(j4[0:16])
        sw1[:, 512:640] = wideN(i4[0:16])
        sw1[:, 640:656] = onehot
        sw1[:, 656:672] = j4[16].reshape(128, 16)
        sw1[:, 672:680] = i4[16].reshape(128, 8)
        sw1[:, 680:712] = wideN(np.ascontiguousarray(ldc[0:16]))
        sw1[:, 712:714] = ldc[16].reshape(128, 2)
        sw1[:, 714:730] = lwc[16].reshape(128, 16)
        sw1[:, 730:746] = wideN(ju.view(np.float16))
        sw1[:, 746:754] = wideN(iu.view(np.float16))
        sw1[:, 754:802] = wideN(j3)
        sw1[:, 802:826] = wideN(i3)
        sw1[:, 832:1088] = wideN(cjhl).view(np.uint8).view(np.float16)
        sw1[:, 1088:1120] = wideN(np.ascontiguousarray(ld8[2:18])
                                  ).view(np.uint8).view(np.float16)
        sw1[:, 1120:1124] = wideN(np.ascontiguousarray(ld8[0:2])
                                  ).view(np.uint8).view(np.float16)
        sw1[:, 1124:1124 + W * 16] = np.tile(lwc[16:17], (1, W)).reshape(128, W * 16)
        sw2b = np.concatenate([wg2_all, wideN(np.ascontiguousarray(lwc[0:16])),
                               np.zeros((128, 2048 - W * 256 - 256), np.float16)],
                              axis=1)
        in_maps.append({
            "sw1": sw1,
            "sw2a": np.ascontiguousarray(wg1_all),
            "sw2b": np.ascontiguousarray(sw2b),
        })
    return in_maps, sigmas, W


def _assemble(res, sigmas):
    out = np.empty((1, N, N, H), np.float32)
    for c in range(NCORES):
        dev = np.asarray(res.results[c]["outp"]).astype(np.float32)
        dev = dev.reshape(IB, NJT, 128 * H)
        blk = out[0, c * IB:(c + 1) * IB].reshape(IB, NJT, 128 * H)
        blk[:, sigmas[c], :] = dev
    return out


def kernel(**inputs):
    from concourse import bass_utils
    in_maps, sigmas, W = make_in_maps(inputs)
    nc = _build_program(W)
    res = bass_utils.run_bass_kernel_spmd(nc, in_maps, core_ids=list(range(NCORES)))
    return _assemble(res, sigmas)


# revision 48
# speedup vs baseline: 1.1537x; 1.1537x over previous
"""Trainium2 Bass kernel for nn_CoordinateConditioning.

out[i,j,h] = v[i,j]*( (X[i]-X[j])@Wcoord[h] + Wdist[h]*R[i,j] + B*Wmask[h] )
             + C[i,h] + C[j,h]
with X = sum_b coords[b], R[i,j] = sum_b 1/(1+||x_b[i]-x_b[j]||^2),
v = pad/uid mask, C = B*c0 + gathered s_to_c sum.

Key structure exploited: ref_space_uid is sorted, so v[i,j] is a narrow
block-diagonal band.  For each core (256 i-rows) only W (~3) of the 16
j-tiles can contain v!=0 pairs; host computes the per-core window-tile
list from the actual inputs and the device program processes exactly
W "window" tiles (full geometry pipeline) + 16-W "dense" tiles
(out = C_i + C_j only, K=17 matmul with a shared stationary operand).
Per-core j-tile order is a host-chosen permutation sigma_c; the host
inverse-permutes the j-tile blocks when assembling the full output.

DMA layout: TRN2 SDMA engines only spray a transfer across all 16
engines when one side spans ~128 partitions; partition-narrow loads
(17/97 rows) land on ONE engine (~22 GB/s).  So the big constant
patterns are staged to SBUF as [128, f] HWDGE loads (sprayed), then
redistributed SBUF->SBUF with 128-partition sources via gpsimd SWDGE
(also sprayed).  Remaining tiny narrow loads ride the Act HWDGE ring so
their single-engine backlog cannot stall the SP ring that carries the
output stores.

Output is written fp16 (rel tolerance is 2e-2; fp16 rounding ~5e-4),
halving the dominant out-DMA traffic.  PSUM->SBUF copies are split
between ACT and DVE.
"""

import numpy as np
import ml_dtypes
from contextlib import ExitStack

FP8 = ml_dtypes.float8_e4m3fn


def _fp8_hi_lo(v):
    """fp8e4m3 hi/lo split: v ~ hi + lo with |err| <= |v| * 2^-8."""
    v = np.ascontiguousarray(v, dtype=np.float32)
    hi = v.astype(FP8)
    lo = (v - hi.astype(np.float32)).astype(FP8)
    return hi, lo

B, N, T, TOKEN_S, DIM_F, H = 4, 2048, 256, 384, 256, 16
NCORES = 8
IB = N // NCORES          # 256 i rows per core
NJT = N // 128            # 16 j tiles
KB = 17                   # K rows per batch for the r2 matmul
KU = 2                    # uid delta rows
KD = 4                    # rows per D_k
KF = 4 * KB + KU + 3 * KD # 82 total J/I feature rows
BIGM = 4096.0

_CACHE = {}


def _split_hi_lo(v):
    """fp16-exact hi/lo split (hi keeps 10 mantissa bits)."""
    v = np.ascontiguousarray(v, dtype=np.float32)
    hi = (v.view(np.uint32) & np.uint32(0xFFFFE000)).view(np.float32)
    return hi, (v - hi).astype(np.float32)


def _host_tables(inputs):
    I = {k: np.asarray(v) for k, v in inputs.items()}
    x = np.ascontiguousarray(I['atom_coords_noisy'], dtype=np.float32)  # [B,N,3]
    m = I['atom_pad_mask'].reshape(-1).astype(np.float32)               # [N]
    uid = I['ref_space_uid'].reshape(-1).astype(np.float32)             # [N]

    # ---- small linears (replicated) ----
    def ln(v, g, b, eps=1e-5):
        mu = v.mean(-1, keepdims=True)
        var = ((v - mu) ** 2).mean(-1, keepdims=True)
        return (v - mu) / np.sqrt(var + eps) * g + b

    s = np.concatenate([I['s_trunk'], I['s_inputs']], -1).astype(np.float32) @ I['W_single'].T
    fe = np.cos(2 * np.pi * (I['times'][:, None] * I['Wf'][:, 0][None, :] + I['bf'])).astype(np.float32)
    s = s + (ln(fe, I['ln_f_g'], I['ln_f_b']) @ I['Wf2s'].T)[:, None, :]
    s2c = ln(s, I['ln_s_g'], I['ln_s_b']) @ I['Wsc'].T                  # [B,T,1]
    ssum = s2c[:, :, 0].sum(0)                                          # [T]
    tok = I['atom_to_token_idx'].reshape(-1).astype(np.int64)
    S = ssum[tok]                                                       # [N]
    af = np.concatenate([I['ref_pos'][0], I['ref_charge'][0][:, None],
                         I['ref_element'][0]], -1).astype(np.float32)   # [N,132]
    c0 = af @ I['Wa'].T + I['ba']                                       # [N,16]
    C = (B * c0 + S[:, None]).astype(np.float32)                        # [N,16]

    X = x.sum(0)                                                        # [N,3]
    Wc = np.asarray(I['Wcoord'], np.float32)                            # [16,3]
    # device feature maps hold X_j - X_i, the formula needs X_i - X_j -> negate
    wtab = np.stack([-Wc[:, 0], -Wc[:, 1], -Wc[:, 2],
                     np.asarray(I['Wdist'], np.float32)[:, 0],
                     B * np.asarray(I['Wmask'], np.float32)[:, 0]], 0)  # [5,16]

    # ---- J/I feature tables for the per-(j,i) matmuls ----
    n2 = np.einsum('bnk,bnk->bn', x.astype(np.float64), x.astype(np.float64)).astype(np.float32)
    ones = np.ones(N, np.float32)
    jf = np.zeros((KF, N), np.float32)
    itab = np.zeros((KF, N), np.float32)
    for b in range(B):
        r = b * KB
        for k in range(3):
            xh, xl = _split_hi_lo(x[b, :, k])
            jf[r + 4 * k + 0] = xh
            jf[r + 4 * k + 1] = xh
            jf[r + 4 * k + 2] = xl
            jf[r + 4 * k + 3] = xl
            itab[r + 4 * k + 0] = -2.0 * xh
            itab[r + 4 * k + 1] = -2.0 * xl
            itab[r + 4 * k + 2] = -2.0 * xh
            itab[r + 4 * k + 3] = -2.0 * xl
        nh, nl = _split_hi_lo(n2[b])
        jf[r + 12], jf[r + 13] = nh, nl
        itab[r + 12], itab[r + 13] = ones, ones
        jf[r + 14], jf[r + 15] = ones, ones
        itab[r + 14], itab[r + 15] = nh, nl
        jf[r + 16] = ones
        itab[r + 16] = ones
    ru = 4 * KB
    jf[ru] = uid + BIGM * (1.0 - m)
    itab[ru] = ones
    jf[ru + 1] = ones
    itab[ru + 1] = -uid + BIGM * (1.0 - m)
    for k in range(3):
        r = ru + KU + KD * k
        Xh, Xl = _split_hi_lo(X[:, k])
        jf[r + 0], jf[r + 1] = Xh, Xl
        itab[r + 0], itab[r + 1] = ones, ones
        jf[r + 2], jf[r + 3] = ones, ones
        itab[r + 2], itab[r + 3] = -Xh, -Xl

    # ---- constant rhs pattern rows (per 256-col jsub block) ----
    # rows 0..79: geometry block-diag(delta_jp * wtab[t]); 80..95: delta_h
    blk = np.zeros((96, 256), np.float32)
    for t in range(5):
        for jp in range(16):
            blk[t * 16 + jp, jp * 16:(jp + 1) * 16] = wtab[t]
    for hp in range(16):
        blk[80 + hp, hp::16] = 1.0
    pc = np.tile(blk, (1, 8))                                           # [96, 2048]

    # bake the per-tile column permutation p -> j = (p%8)*16 + p//8 into jf
    # so device lhsT slices are plain contiguous (walrus: one free dim only)
    p = np.arange(128)
    perm = (np.arange(N) // 128) * 128 + ((p % 8) * 16 + p // 8)[np.tile(p, N // 128) * 0 + np.arange(N) % 128]
    jf = np.ascontiguousarray(jf[:, perm])

    cflat = C.reshape(1, N * H).astype(np.float32)
    return jf, itab, pc, C, cflat, m.astype(bool), uid


def _windows(m, uid):
    """Per-core window j-tile lists (tiles that can hold v!=0 pairs) and
    the per-core slot->j-tile permutation sigma (window tiles first)."""
    tiles_per_core = []
    for c in range(NCORES):
        sl = slice(c * IB, (c + 1) * IB)
        vi = m[sl]
        if vi.any():
            U = np.unique(uid[sl][vi])
            pj = np.where(m & np.isin(uid, U))[0]
            tiles = sorted(set((pj // 128).tolist()))
        else:
            tiles = []
        tiles_per_core.append(tiles)
    W = max(1, max(len(t) for t in tiles_per_core))
    sigmas = []
    for tiles in tiles_per_core:
        rest = [t for t in range(NJT) if t not in tiles]
        pad = rest[:W - len(tiles)]
        rest2 = rest[W - len(tiles):]
        sigmas.append(np.array(tiles + pad + rest2, np.int64))
    return sigmas, W


def _build_program(W):
    key = ('nc', W)
    if key in _CACHE:
        return _CACHE[key]
    import concourse.bass as bass
    import concourse.bacc as bacc
    import concourse.tile as tile
    from concourse import mybir

    f32 = mybir.dt.float32
    f32r = mybir.dt.float32r
    f16 = mybir.dt.float16
    f8 = mybir.dt.float8e4

    GW = W * 2048            # valid geometry columns in the merged tile

    nc = bacc.Bacc("TRN2", target_bir_lowering=False, debug=False)
    # wide staging payloads (all sprayed as [128, f] loads, then SWDGE-
    # redistributed with 128-partition sources into the narrow operand
    # layouts).  sw1 feeds the dense steps + stage 1, sw2 the window steps.
    # The delta_h pattern is generated on device (17 strided memsets), not
    # loaded: it would otherwise dominate the dense-path critical DMA.
    #   sw1 = [ Cjflat(256) | J4 rows0-15(256) | I4 rows0-15(128) | delta16(16) | pad ]
    #   sw2a = [ geo rows0-63 (W*1024) ]   (8KB rows, the proven-fast shape)
    #   sw2b = [ geo rows64-79 (W*256) | lwc rows0-15(256) | pad ]
    FW1 = 2048
    sw1 = nc.dram_tensor("sw1", [128, FW1], f16, kind="ExternalInput").ap()
    sw2a = nc.dram_tensor("sw2a", [128, W * 1024], f16, kind="ExternalInput").ap()
    FW2B = 2048
    sw2b = nc.dram_tensor("sw2b", [128, FW2B], f16, kind="ExternalInput").ap()
    outp = nc.dram_tensor("outp", [IB, N * H], f16, kind="ExternalOutput").ap()

    with tile.TileContext(nc) as tc:
        with ExitStack() as ctx:
            cpool = ctx.enter_context(tc.tile_pool(name="const", bufs=1))
            J4 = cpool.tile([KB, B * W * 128], f16, tag="J4")
            I4 = cpool.tile([KB, B * IB], f16, tag="I4")
            J3 = cpool.tile([KD, 3 * W * 128], f16, tag="J3")
            I3 = cpool.tile([KD, 3 * IB], f16, tag="I3")
            Ju = cpool.tile([KU, W * 128], f32r, tag="Ju")
            Iu = cpool.tile([KU, IB], f32r, tag="Iu")
            # window rhs pattern tile (fp16): rows 0..16 = [Cj; delta_h],
            # rows 17..96 = geometry; only window columns (< GW) are used
            PB = cpool.tile([97, GW], f16, tag="PB")
            # dense rhs pattern tile (fp8e4, DoubleRow): row 0 = Cj_hi,
            # row 1 = Cj_lo, rows 2..17 = delta_h; the DoubleRow pair dim is
            # stride-0 on this operand
            PB8 = cpool.tile([18, NJT * 2048], f8, tag="PB8")
            # dense lhsT fp8 pairs: row 0/1 = (1,0), rows 2..17 = (CiH, CiL)
            LD8 = cpool.tile([18, 2 * IB], f8, tag="LD8")
            # window lhsT helper: row 0 = ones, rows 1..17 = Ci^T
            Ld = cpool.tile([17, IB], f16, tag="Ld")
            # staging tiles for the wide loads (sprayed across all 16 SDMA
            # engines), redistributed below with 128-partition sources
            SW1 = cpool.tile([128, FW1], f16, tag="SW1")
            SW2A = cpool.tile([128, W * 1024], f16, tag="SW2A")
            SW2B = cpool.tile([128, FW2B], f16, tag="SW2B")
            SD = cpool.tile([128, 1024], f16, tag="SD")
            SD8 = cpool.tile([128, 4096], f8, tag="SD8")

            # wide staged loads on the SP HWDGE ring (shared with stores)
            nc.sync.dma_start(SW1[:, :], sw1[:, :])
            nc.sync.dma_start(SW2A[:, :], sw2a[:, :])
            nc.sync.dma_start(SW2B[:, :], sw2b[:, :])

            # delta_h staging, generated on device: partition p holds the
            # 16-periodic one-hot(p//8) pattern, expanded by DVE broadcast
            # (fp16 copy for the window tile, fp8 cast for the dense tile)
            nc.vector.tensor_copy(
                SD[:, :].rearrange("p (a s) -> p a s", s=16),
                SW1[:, 640:656].unsqueeze(1).broadcast_to([128, 64, 16]))
            nc.vector.tensor_copy(
                SD8[:, :].rearrange("p (a s) -> p a s", s=16),
                SW1[:, 640:656].unsqueeze(1).broadcast_to([128, 256, 16]))

            # Everything is SWDGE-redistributed out of the wide staging
            # tiles; there are no partition-narrow DRAM loads at all (they
            # hot-spot one SDMA engine and gate every completion).  SWDGE
            # assigns each dma_start to ONE SDMA engine (~25 GB/s), so the
            # big patterns move as ~128KB pieces across several
            # instructions.  Order tracks consumption: dense-path operands
            # first, stage-1 operands, then the window-path operands.
            def chunk16(c):
                nc.gpsimd.dma_start(
                    PB[0:1, c * 4096:(c + 1) * 4096].rearrange(
                        "r (q f) -> r q f", q=16),
                    SW1[c * 16:(c + 1) * 16, 0:256])
                nc.gpsimd.dma_start(
                    PB[1:17, c * 4096:(c + 1) * 4096].rearrange(
                        "r (q f) -> r q f", q=8),
                    SD[:, c * 512:(c + 1) * 512])

            def chunk8(c):
                nc.gpsimd.dma_start(
                    PB8[2:18, c * 4096:(c + 1) * 4096].rearrange(
                        "r (q f) -> r q f", q=8),
                    SD8[:, c * 512:(c + 1) * 512])

            nc.gpsimd.dma_start(
                LD8[2:18, :].rearrange("r (q f) -> r q f", q=8),
                SW1[:, 1088:1120].bitcast(f8))
            nc.gpsimd.dma_start(
                LD8[0:2, :].rearrange("r (q f) -> r q f", q=64),
                SW1[:, 1120:1124].bitcast(f8))
            nc.gpsimd.dma_start(
                PB8[0:2, :].rearrange("r (q f) -> r q f", q=64),
                SW1[:, 832:1088].bitcast(f8))
            # first dense steps gate on these; small pieces complete in
            # ~1.5-3us each on their SWDGE engines
            nc.gpsimd.dma_start(
                PB8[2:18, 8192:10240].rearrange("r (q f) -> r q f", q=8),
                SD8[:, 1024:1280])
            nc.gpsimd.dma_start(
                PB8[2:18, 10240:12288].rearrange("r (q f) -> r q f", q=8),
                SD8[:, 1280:1536])
            nc.gpsimd.dma_start(
                PB8[2:18, 12288:16384].rearrange("r (q f) -> r q f", q=8),
                SD8[:, 1536:2048])
            nc.gpsimd.dma_start(
                J4[0:16, :].rearrange("r (q f) -> r q f", q=8),
                SW1[:, 256:512])
            nc.gpsimd.dma_start(
                I4[0:16, :].rearrange("r (q f) -> r q f", q=8),
                SW1[:, 512:640])
            nc.gpsimd.dma_start(
                J4[16:17, :].rearrange("r (q f) -> r q f", q=128),
                SW1[:, 656:672])
            nc.gpsimd.dma_start(
                I4[16:17, :].rearrange("r (q f) -> r q f", q=128),
                SW1[:, 672:680])
            nc.gpsimd.dma_start(
                Ju[:, :].bitcast(f16).rearrange("r (q f) -> r q f", q=64),
                SW1[:, 730:746])
            nc.gpsimd.dma_start(
                Iu[:, :].bitcast(f16).rearrange("r (q f) -> r q f", q=64),
                SW1[:, 746:754])
            nc.gpsimd.dma_start(
                J3[:, :].rearrange("r (q f) -> r q f", q=32),
                SW1[:, 754:802])
            nc.gpsimd.dma_start(
                I3[:, :].rearrange("r (q f) -> r q f", q=32),
                SW1[:, 802:826])
            chunk8(4)
            chunk8(5)
            # window lhsT per slot: rows 0..17 = [ones; Ci], 17..97 geometry (repack)
            Lw = []
            for s in range(W):
                Lt = cpool.tile([97, 2048], f16, tag=f"Lw{s}")
                nc.gpsimd.dma_start(
                    Lt[0:16, :].rearrange("r (q f) -> r q f", q=8),
                    SW2B[:, W * 256:W * 256 + 256])
                nc.gpsimd.dma_start(
                    Lt[16:17, :].rearrange("r (q f) -> r q f", q=128),
                    SW1[:, 714:730])
                Lw.append(Lt)
            chunk8(6)
            chunk8(7)
            nc.gpsimd.dma_start(
                PB[17:81, 0:GW].rearrange("r (q f) -> r q f", q=2),
                SW2A[:, :])
            nc.gpsimd.dma_start(
                PB[81:97, 0:GW].rearrange("r (q f) -> r q f", q=8),
                SW2B[:, 0:W * 256])
            chunk16(0)
            chunk16(1)

            psO = ctx.enter_context(tc.tile_pool(name="psO", bufs=4, space="PSUM"))
            wk = ctx.enter_context(tc.tile_pool(name="wk", bufs=2))
            wkr = ctx.enter_context(tc.tile_pool(name="wkr", bufs=4))
            stg = ctx.enter_context(tc.tile_pool(name="stg", bufs=5))
            dstv = outp.rearrange("(t p) nh -> p t nh", t=2)

            def emit_step(k, slot):
                """One output step: 8 (dense) or 16 (window) matmuls ->
                4 PSUM->SBUF fp16 copies -> one 1MB store."""
                dense = slot >= W
                # while stage-1's chain owns the DVE queue, ACT takes all
                # 4 PSUM->SBUF copies; otherwise split 2/2
                a = 3 if 4 <= k < 9 else 2
                qidx = 0
                st = stg.tile([128, 4096], f16, tag="st", name="st")
                for it in range(2):
                    for g in range(2):
                        po = psO.tile([128, 1024], f32, tag="po", name="po")
                        if dense:
                            for jl in range(2):
                                c0 = slot * 2048 + g * 1024 + jl * 512
                                nc.tensor.matmul(
                                    po[:, jl * 512:(jl + 1) * 512],
                                    LD8[0:18, :].rearrange(
                                        "k (two m) -> k two m",
                                        two=2)[:, :, it * 128:(it + 1) * 128],
                                    PB8[0:18, c0:c0 + 512].unsqueeze(1)
                                        .broadcast_to([18, 2, 512]),
                                    start=True, stop=True,
                                    perf_mode=mybir.MatmulPerfMode.DoubleRow)
                        else:
                            for jl in range(4):
                                js = g * 4 + jl
                                base = js * 256 + it * 128
                                nc.tensor.matmul(
                                    po[:, jl * 256:(jl + 1) * 256],
                                    Lw[slot][0:97, base:base + 128],
                                    PB[0:97, slot * 2048 + js * 256:slot * 2048 + (js + 1) * 256],
                                    start=True, stop=True)
                        dst = st[:, it * 2048 + g * 1024:it * 2048 + (g + 1) * 1024]
                        if qidx < a:
                            nc.scalar.copy(dst, po[:, :])
                        else:
                            nc.vector.tensor_copy(dst, po[:, :])
                        qidx += 1
                nc.sync.dma_start(
                    dstv[:, :, slot * 2048:(slot + 1) * 2048],
                    st[:, :].rearrange("p (t c) -> p t c", t=2))

            def emit_stage1():
                """Geometry features for the W window slots."""
                for s in range(W):
                    ps1 = psO.tile([128, 1024], f32, tag="po", name="ps1")
                    for b in range(B):
                        nc.tensor.matmul(ps1[:, b * 256:(b + 1) * 256],
                                         J4[:, (b * W + s) * 128:(b * W + s + 1) * 128],
                                         I4[:, b * IB:(b + 1) * IB],
                                         start=True, stop=True)
                    rc = wkr.tile([128, 1024], f32, tag="rc", name="rc")
                    nc.vector.reciprocal_approx_fast(rc[:, :], ps1[:, :])
                    ps2 = psO.tile([128, 1024], f32, tag="po", name="ps2")
                    nc.tensor.matmul(ps2[:, 0:256],
                                     Ju[:, s * 128:(s + 1) * 128], Iu[:, :],
                                     start=True, stop=True)
                    for k in range(3):
                        nc.tensor.matmul(ps2[:, 256 + k * 256:512 + k * 256],
                                         J3[:, (k * W + s) * 128:(k * W + s + 1) * 128],
                                         I3[:, k * IB:(k + 1) * IB],
                                         start=True, stop=True)
                    # drain ps2 to SBUF immediately (ACT) so the PSUM bank
                    # frees without waiting for the whole DVE chain; the PE
                    # can then roll straight into the next output step
                    pv = wkr.tile([128, 1024], f32, tag="pv", name="pv")
                    nc.scalar.copy(pv[:, :], ps2[:, :])
                    r2h = wk.tile([128, 512], f32, tag="r2h", name="r2h")
                    Rt = wk.tile([128, 256], f32, tag="Rt", name="Rt")
                    # gpsimd stays DMA-only (tensor ops there force a 14us
                    # ucode LOAD_LIB swap mid-kernel)
                    nc.vector.tensor_add(r2h[:, :], rc[:, 0:512], rc[:, 512:1024])
                    nc.vector.tensor_add(Rt[:, :], r2h[:, 0:256], r2h[:, 256:512])

                    F5 = wk.tile([128, 1280], f16, tag="F5", name="F5")
                    vt = wk.tile([128, 256], f32, tag="vt", name="vt")
                    nc.vector.tensor_scalar(vt[:, :], pv[:, 0:256], 0.0, None,
                                            op0=mybir.AluOpType.is_equal)
                    for k in range(3):
                        nc.vector.tensor_mul(F5[:, k * 256:(k + 1) * 256],
                                             vt[:, :], pv[:, 256 + k * 256:512 + k * 256])
                    nc.vector.tensor_mul(F5[:, 768:1024], vt[:, :], Rt[:, :])
                    nc.vector.tensor_copy(F5[:, 1024:1280], vt[:, :])

                    # repack: permuted partitions make each feature's src contiguous
                    for t in range(5):
                        dst = Lw[s][17 + t * 16:17 + (t + 1) * 16, :].rearrange(
                            "k (a i) -> k a i", a=8)
                        nc.gpsimd.dma_start(dst, F5[:, t * 256:(t + 1) * 256])

            # ---- emission order: a few dense steps first (their operands
            # arrive earliest), then stage 1, then the rest with the
            # windows interleaved ----
            dense_list = list(range(W, NJT))
            for k, slot in enumerate(dense_list[:4]):
                emit_step(k, slot)
            emit_stage1()
            post = dense_list[4:9]
            for w in range(W):
                post.append(w)
                if 9 + w < len(dense_list):
                    post.append(dense_list[9 + w])
            for k, slot in enumerate(post):
                emit_step(4 + k, slot)
    nc.compile()
    _CACHE[key] = nc
    return nc


def make_in_maps(inputs):
    jf, itab, pc, C, cflat, m, uid = _host_tables(inputs)
    sigmas, W = _windows(m, uid)
    pcw16 = pc.astype(np.float16)             # [96, 2048]
    pcd16 = np.ascontiguousarray(pc[80:96]).astype(np.float16)  # [16, 2048] delta
    in_maps = []
    ru = 4 * KB
    # geometry rows tiled x W slots: [80, W*2048]
    geo = np.tile(pcw16[0:80], (1, W))
    wg1_all = np.ascontiguousarray(geo[0:64]).reshape(64, 2, W * 1024).reshape(128, W * 1024)
    wg2_all = np.ascontiguousarray(geo[64:80]).reshape(16, 8, W * 256).reshape(128, W * 256)

    def wideN(a):
        """[R, F] -> [128, R*F//128] staging layout (row r piece q -> part
        r*(128//R)+q), for R dividing 128."""
        r, f = a.shape
        q = 128 // r
        return np.ascontiguousarray(a).reshape(r, q, f // q).reshape(128, f // q)

    for c in range(NCORES):
        sl = slice(c * IB, (c + 1) * IB)
        sg = sigmas[c]
        jfw = np.concatenate([jf[:, t * 128:(t + 1) * 128] for t in sg[:W]], axis=1)
        cflP = np.concatenate([cflat[:, t * 2048:(t + 1) * 2048] for t in sg], axis=1)
        citc = np.ascontiguousarray(C.T[:, sl]).astype(np.float16)   # [16, IB]
        cf16 = np.ascontiguousarray(cflP).astype(np.float16)
        ldc = np.concatenate([np.ones((1, IB), np.float16), citc], 0)
        lwc = np.concatenate([np.ones((1, 2048), np.float16),
                              np.tile(citc, (1, 8))], 0)
        ifc = itab[:, sl]
        j4 = np.concatenate([jfw[b * KB:(b + 1) * KB] for b in range(B)],
                            axis=1).astype(np.float16)
        i4 = np.concatenate([ifc[b * KB:(b + 1) * KB] for b in range(B)],
                            axis=1).astype(np.float16)
        j3 = np.concatenate([jfw[ru + KU + KD * k:ru + KU + KD * (k + 1)]
                             for k in range(3)], axis=1).astype(np.float16)
        i3 = np.concatenate([ifc[ru + KU + KD * k:ru + KU + KD * (k + 1)]
                             for k in range(3)], axis=1).astype(np.float16)
        ju = np.ascontiguousarray(jfw[ru:ru + KU], np.float32)   # f32r bytes
        iu = np.ascontiguousarray(ifc[ru:ru + KU], np.float32)
        onehot = np.zeros((128, 16), np.float16)
        onehot[np.arange(128), np.arange(128) // 8] = 1.0
        # fp8 dense operands: Cj/Ci hi+lo planes (DoubleRow pairs)
        cjh, cjl = _fp8_hi_lo(cflP)                     # [1, 32768] each
        cjhl = np.concatenate([cjh, cjl], 0)            # [2, 32768] fp8
        cif = np.ascontiguousarray(C.T[:, sl], np.float32)  # [16, IB]
        cih, cil = _fp8_hi_lo(cif)
        ld8 = np.zeros((18, 2, IB), FP8)
        ld8[0, 0] = FP8(1.0)
        ld8[1, 0] = FP8(1.0)
        ld8[2:, 0] = cih
        ld8[2:, 1] = cil
        ld8 = ld8.reshape(18, 2 * IB)
        # one wide staging tensor holds every operand; segment offsets must
        # match the device-side redistributes in _build_program
        sw1 = np.zeros((128, 2048), np.float16)
        sw1[:, 0:256] = cf16.reshape(128, 256)
        sw1[:, 256:512] = wideN(j4[0:16])
        sw1[:, 512:640] = wideN(i4[0:16])
        sw1[:, 640:656] = onehot
        sw1[:, 656:672] = j4[16].reshape(128, 16)
        sw1[:, 672:680] = i4[16].reshape(128, 8)
        sw1[:, 680:712] = wideN(np.ascontiguousarray(ldc[0:16]))
        sw1[:, 712:714] = ldc[16].reshape(128, 2)
        sw1[:, 714:730] = lwc[16].reshape(128, 16)
        sw1[:, 730:746] = wideN(ju.view(np.float16))
        sw1[:, 746:754] = wideN(iu.view(np.float16))
        sw1[:, 754:802] = wideN(j3)
        sw1[:, 802:826] = wideN(i3)
        sw1[:, 832:1088] = wideN(cjhl).view(np.uint8).view(np.float16)
        sw1[:, 1088:1120] = wideN(np.ascontiguousarray(ld8[2:18])
                                  ).view(np.uint8).view(np.float16)
        sw1[:, 1120:1124] = wideN(np.ascontiguousarray(ld8[0:2])
                                  ).view(np.uint8).view(np.float16)
        sw1[:, 1124:1124 + W * 16] = np.tile(lwc[16:17], (1, W)).reshape(128, W * 16)
        sw2b = np.concatenate([wg2_all, wideN(np.ascontiguousarray(lwc[0:16])),
                               np.zeros((128, 2048 - W * 256 - 256), np.float16)],
                              axis=1)
        in_maps.append({
            "sw1": sw1,
            "sw2a": np.ascontiguousarray(wg1_all),
            "sw2b": np.ascontiguousarray(sw2b),
        })
    return in_maps, sigmas, W


def _assemble(res, sigmas):
    out = np.empty((1, N, N, H), np.float32)
    for c in range(NCORES):
        dev = np.asarray(res.results[c]["outp"]).astype(np.float32)
        dev = dev.reshape(IB, NJT, 128 * H)
        blk = out[0, c * IB:(c + 1) * IB].reshape(IB, NJT, 128 * H)
        blk[:, sigmas[c], :] = dev
    return out


def kernel(**inputs):
    from concourse import bass_utils
    in_maps, sigmas, W = make_in_maps(inputs)
    nc = _build_program(W)
    res = bass_utils.run_bass_kernel_spmd(nc, in_maps, core_ids=list(range(NCORES)))
    return _assemble(res, sigmas)


# revision 49
# speedup vs baseline: 1.1887x; 1.0303x over previous
"""Trainium2 Bass kernel for nn_CoordinateConditioning.

out[i,j,h] = v[i,j]*( (X[i]-X[j])@Wcoord[h] + Wdist[h]*R[i,j] + B*Wmask[h] )
             + C[i,h] + C[j,h]
with X = sum_b coords[b], R[i,j] = sum_b 1/(1+||x_b[i]-x_b[j]||^2),
v = pad/uid mask, C = B*c0 + gathered s_to_c sum.

Key structure exploited: ref_space_uid is sorted, so v[i,j] is a narrow
block-diagonal band.  For each core (256 i-rows) only W (~3) of the 16
j-tiles can contain v!=0 pairs; host computes the per-core window-tile
list from the actual inputs and the device program processes exactly
W "window" tiles (full geometry pipeline) + 16-W "dense" tiles
(out = C_i + C_j only, K=17 matmul with a shared stationary operand).
Per-core j-tile order is a host-chosen permutation sigma_c; the host
inverse-permutes the j-tile blocks when assembling the full output.

DMA layout: TRN2 SDMA engines only spray a transfer across all 16
engines when one side spans ~128 partitions; partition-narrow loads
(17/97 rows) land on ONE engine (~22 GB/s).  So the big constant
patterns are staged to SBUF as [128, f] HWDGE loads (sprayed), then
redistributed SBUF->SBUF with 128-partition sources via gpsimd SWDGE
(also sprayed).  Remaining tiny narrow loads ride the Act HWDGE ring so
their single-engine backlog cannot stall the SP ring that carries the
output stores.

Output is written fp16 (rel tolerance is 2e-2; fp16 rounding ~5e-4),
halving the dominant out-DMA traffic.  PSUM->SBUF copies are split
between ACT and DVE.
"""

import numpy as np
import ml_dtypes
from contextlib import ExitStack

FP8 = ml_dtypes.float8_e4m3fn


def _fp8_hi_lo(v):
    """fp8e4m3 hi/lo split: v ~ hi + lo with |err| <= |v| * 2^-8."""
    v = np.ascontiguousarray(v, dtype=np.float32)
    hi = v.astype(FP8)
    lo = (v - hi.astype(np.float32)).astype(FP8)
    return hi, lo

B, N, T, TOKEN_S, DIM_F, H = 4, 2048, 256, 384, 256, 16
NCORES = 8
IB = N // NCORES          # 256 i rows per core
NJT = N // 128            # 16 j tiles
KB = 17                   # K rows per batch for the r2 matmul
KU = 2                    # uid delta rows
KD = 4                    # rows per D_k
KF = 4 * KB + KU + 3 * KD # 82 total J/I feature rows
BIGM = 4096.0

_CACHE = {}


def _split_hi_lo(v):
    """fp16-exact hi/lo split (hi keeps 10 mantissa bits)."""
    v = np.ascontiguousarray(v, dtype=np.float32)
    hi = (v.view(np.uint32) & np.uint32(0xFFFFE000)).view(np.float32)
    return hi, (v - hi).astype(np.float32)


def _host_tables(inputs):
    I = {k: np.asarray(v) for k, v in inputs.items()}
    x = np.ascontiguousarray(I['atom_coords_noisy'], dtype=np.float32)  # [B,N,3]
    m = I['atom_pad_mask'].reshape(-1).astype(np.float32)               # [N]
    uid = I['ref_space_uid'].reshape(-1).astype(np.float32)             # [N]

    # ---- small linears (replicated) ----
    def ln(v, g, b, eps=1e-5):
        mu = v.mean(-1, keepdims=True)
        var = ((v - mu) ** 2).mean(-1, keepdims=True)
        return (v - mu) / np.sqrt(var + eps) * g + b

    s = np.concatenate([I['s_trunk'], I['s_inputs']], -1).astype(np.float32) @ I['W_single'].T
    fe = np.cos(2 * np.pi * (I['times'][:, None] * I['Wf'][:, 0][None, :] + I['bf'])).astype(np.float32)
    s = s + (ln(fe, I['ln_f_g'], I['ln_f_b']) @ I['Wf2s'].T)[:, None, :]
    s2c = ln(s, I['ln_s_g'], I['ln_s_b']) @ I['Wsc'].T                  # [B,T,1]
    ssum = s2c[:, :, 0].sum(0)                                          # [T]
    tok = I['atom_to_token_idx'].reshape(-1).astype(np.int64)
    S = ssum[tok]                                                       # [N]
    af = np.concatenate([I['ref_pos'][0], I['ref_charge'][0][:, None],
                         I['ref_element'][0]], -1).astype(np.float32)   # [N,132]
    c0 = af @ I['Wa'].T + I['ba']                                       # [N,16]
    C = (B * c0 + S[:, None]).astype(np.float32)                        # [N,16]

    X = x.sum(0)                                                        # [N,3]
    Wc = np.asarray(I['Wcoord'], np.float32)                            # [16,3]
    # device feature maps hold X_j - X_i, the formula needs X_i - X_j -> negate
    wtab = np.stack([-Wc[:, 0], -Wc[:, 1], -Wc[:, 2],
                     np.asarray(I['Wdist'], np.float32)[:, 0],
                     B * np.asarray(I['Wmask'], np.float32)[:, 0]], 0)  # [5,16]

    # ---- J/I feature tables for the per-(j,i) matmuls ----
    n2 = np.einsum('bnk,bnk->bn', x.astype(np.float64), x.astype(np.float64)).astype(np.float32)
    ones = np.ones(N, np.float32)
    jf = np.zeros((KF, N), np.float32)
    itab = np.zeros((KF, N), np.float32)
    for b in range(B):
        r = b * KB
        for k in range(3):
            xh, xl = _split_hi_lo(x[b, :, k])
            jf[r + 4 * k + 0] = xh
            jf[r + 4 * k + 1] = xh
            jf[r + 4 * k + 2] = xl
            jf[r + 4 * k + 3] = xl
            itab[r + 4 * k + 0] = -2.0 * xh
            itab[r + 4 * k + 1] = -2.0 * xl
            itab[r + 4 * k + 2] = -2.0 * xh
            itab[r + 4 * k + 3] = -2.0 * xl
        nh, nl = _split_hi_lo(n2[b])
        jf[r + 12], jf[r + 13] = nh, nl
        itab[r + 12], itab[r + 13] = ones, ones
        jf[r + 14], jf[r + 15] = ones, ones
        itab[r + 14], itab[r + 15] = nh, nl
        jf[r + 16] = ones
        itab[r + 16] = ones
    ru = 4 * KB
    jf[ru] = uid + BIGM * (1.0 - m)
    itab[ru] = ones
    jf[ru + 1] = ones
    itab[ru + 1] = -uid + BIGM * (1.0 - m)
    for k in range(3):
        r = ru + KU + KD * k
        Xh, Xl = _split_hi_lo(X[:, k])
        jf[r + 0], jf[r + 1] = Xh, Xl
        itab[r + 0], itab[r + 1] = ones, ones
        jf[r + 2], jf[r + 3] = ones, ones
        itab[r + 2], itab[r + 3] = -Xh, -Xl

    # ---- constant rhs pattern rows (per 256-col jsub block) ----
    # rows 0..79: geometry block-diag(delta_jp * wtab[t]); 80..95: delta_h
    blk = np.zeros((96, 256), np.float32)
    for t in range(5):
        for jp in range(16):
            blk[t * 16 + jp, jp * 16:(jp + 1) * 16] = wtab[t]
    for hp in range(16):
        blk[80 + hp, hp::16] = 1.0
    pc = np.tile(blk, (1, 8))                                           # [96, 2048]

    # bake the per-tile column permutation p -> j = (p%8)*16 + p//8 into jf
    # so device lhsT slices are plain contiguous (walrus: one free dim only)
    p = np.arange(128)
    perm = (np.arange(N) // 128) * 128 + ((p % 8) * 16 + p // 8)[np.tile(p, N // 128) * 0 + np.arange(N) % 128]
    jf = np.ascontiguousarray(jf[:, perm])

    cflat = C.reshape(1, N * H).astype(np.float32)
    return jf, itab, pc, C, cflat, m.astype(bool), uid


def _windows(m, uid):
    """Per-core window j-tile lists (tiles that can hold v!=0 pairs) and
    the per-core slot->j-tile permutation sigma (window tiles first)."""
    tiles_per_core = []
    for c in range(NCORES):
        sl = slice(c * IB, (c + 1) * IB)
        vi = m[sl]
        if vi.any():
            U = np.unique(uid[sl][vi])
            pj = np.where(m & np.isin(uid, U))[0]
            tiles = sorted(set((pj // 128).tolist()))
        else:
            tiles = []
        tiles_per_core.append(tiles)
    W = max(1, max(len(t) for t in tiles_per_core))
    sigmas = []
    for tiles in tiles_per_core:
        rest = [t for t in range(NJT) if t not in tiles]
        pad = rest[:W - len(tiles)]
        rest2 = rest[W - len(tiles):]
        sigmas.append(np.array(tiles + pad + rest2, np.int64))
    return sigmas, W


def _build_program(W):
    key = ('nc', W)
    if key in _CACHE:
        return _CACHE[key]
    import concourse.bass as bass
    import concourse.bacc as bacc
    import concourse.tile as tile
    from concourse import mybir

    f32 = mybir.dt.float32
    f32r = mybir.dt.float32r
    f16 = mybir.dt.float16
    f8 = mybir.dt.float8e4

    GW = W * 2048            # valid geometry columns in the merged tile

    nc = bacc.Bacc("TRN2", target_bir_lowering=False, debug=False)
    # wide staging payloads (all sprayed as [128, f] loads, then SWDGE-
    # redistributed with 128-partition sources into the narrow operand
    # layouts).  sw1 feeds the dense steps + stage 1, sw2 the window steps.
    # The delta_h pattern is generated on device (17 strided memsets), not
    # loaded: it would otherwise dominate the dense-path critical DMA.
    #   sw1 = [ Cjflat(256) | J4 rows0-15(256) | I4 rows0-15(128) | delta16(16) | pad ]
    #   sw2a = [ geo rows0-63 (W*1024) ]   (8KB rows, the proven-fast shape)
    #   sw2b = [ geo rows64-79 (W*256) | lwc rows0-15(256) | pad ]
    FW1 = 2048
    sw1 = nc.dram_tensor("sw1", [128, FW1], f16, kind="ExternalInput").ap()
    sw2a = nc.dram_tensor("sw2a", [128, W * 1024], f16, kind="ExternalInput").ap()
    FW2B = 2048
    sw2b = nc.dram_tensor("sw2b", [128, FW2B], f16, kind="ExternalInput").ap()
    outp = nc.dram_tensor("outp", [IB, N * H], f16, kind="ExternalOutput").ap()

    with tile.TileContext(nc) as tc:
        with ExitStack() as ctx:
            cpool = ctx.enter_context(tc.tile_pool(name="const", bufs=1))
            J4 = cpool.tile([KB, B * W * 128], f16, tag="J4")
            I4 = cpool.tile([KB, B * IB], f16, tag="I4")
            J3 = cpool.tile([KD, 3 * W * 128], f16, tag="J3")
            I3 = cpool.tile([KD, 3 * IB], f16, tag="I3")
            Ju = cpool.tile([KU, W * 128], f32r, tag="Ju")
            Iu = cpool.tile([KU, IB], f32r, tag="Iu")
            # window rhs pattern tile (fp16): rows 0..16 = [Cj; delta_h],
            # rows 17..96 = geometry; only window columns (< GW) are used
            PB = cpool.tile([97, GW], f16, tag="PB")
            # dense rhs pattern tile (fp8e4, DoubleRow): row 0 = Cj_hi,
            # row 1 = Cj_lo, rows 2..17 = delta_h; the DoubleRow pair dim is
            # stride-0 on this operand
            PB8 = cpool.tile([18, NJT * 2048], f8, tag="PB8")
            # dense lhsT fp8 pairs: row 0/1 = (1,0), rows 2..17 = (CiH, CiL)
            LD8 = cpool.tile([18, 2 * IB], f8, tag="LD8")
            # window lhsT helper: row 0 = ones, rows 1..17 = Ci^T
            Ld = cpool.tile([17, IB], f16, tag="Ld")
            # staging tiles for the wide loads (sprayed across all 16 SDMA
            # engines), redistributed below with 128-partition sources
            SW1 = cpool.tile([128, FW1], f16, tag="SW1")
            SW2A = cpool.tile([128, W * 1024], f16, tag="SW2A")
            SW2B = cpool.tile([128, FW2B], f16, tag="SW2B")
            SD = cpool.tile([128, 1024], f16, tag="SD")
            SD8 = cpool.tile([128, 4096], f8, tag="SD8")

            # wide staged loads on the SP HWDGE ring (shared with stores)
            nc.sync.dma_start(SW1[:, :], sw1[:, :])
            nc.sync.dma_start(SW2A[:, :], sw2a[:, :])
            nc.sync.dma_start(SW2B[:, :], sw2b[:, :])

            # delta_h staging, generated on device: partition p holds the
            # 16-periodic one-hot(p//8) pattern, expanded by DVE broadcast
            # (fp16 copy for the window tile, fp8 cast for the dense tile)
            nc.vector.tensor_copy(
                SD[:, :].rearrange("p (a s) -> p a s", s=16),
                SW1[:, 640:656].unsqueeze(1).broadcast_to([128, 64, 16]))
            nc.vector.tensor_copy(
                SD8[:, :].rearrange("p (a s) -> p a s", s=16),
                SW1[:, 640:656].unsqueeze(1).broadcast_to([128, 256, 16]))

            # Everything is SWDGE-redistributed out of the wide staging
            # tiles; there are no partition-narrow DRAM loads at all (they
            # hot-spot one SDMA engine and gate every completion).  SWDGE
            # assigns each dma_start to ONE SDMA engine (~25 GB/s), so the
            # big patterns move as ~128KB pieces across several
            # instructions.  Order tracks consumption: dense-path operands
            # first, stage-1 operands, then the window-path operands.
            def chunk16(c):
                nc.gpsimd.dma_start(
                    PB[0:1, c * 4096:(c + 1) * 4096].rearrange(
                        "r (q f) -> r q f", q=16),
                    SW1[c * 16:(c + 1) * 16, 0:256])
                nc.gpsimd.dma_start(
                    PB[1:17, c * 4096:(c + 1) * 4096].rearrange(
                        "r (q f) -> r q f", q=8),
                    SD[:, c * 512:(c + 1) * 512])

            def chunk8(c):
                nc.gpsimd.dma_start(
                    PB8[2:18, c * 4096:(c + 1) * 4096].rearrange(
                        "r (q f) -> r q f", q=8),
                    SD8[:, c * 512:(c + 1) * 512])

            nc.gpsimd.dma_start(
                LD8[2:18, :].rearrange("r (q f) -> r q f", q=8),
                SW1[:, 1088:1120].bitcast(f8))
            nc.gpsimd.dma_start(
                LD8[0:2, :].rearrange("r (q f) -> r q f", q=64),
                SW1[:, 1120:1124].bitcast(f8))
            nc.gpsimd.dma_start(
                PB8[0:2, :].rearrange("r (q f) -> r q f", q=64),
                SW1[:, 832:1088].bitcast(f8))
            # first dense steps gate on these; small pieces complete in
            # ~1.5-3us each on their SWDGE engines
            nc.gpsimd.dma_start(
                PB8[2:18, 8192:10240].rearrange("r (q f) -> r q f", q=8),
                SD8[:, 1024:1280])
            nc.gpsimd.dma_start(
                PB8[2:18, 10240:12288].rearrange("r (q f) -> r q f", q=8),
                SD8[:, 1280:1536])
            nc.gpsimd.dma_start(
                PB8[2:18, 12288:16384].rearrange("r (q f) -> r q f", q=8),
                SD8[:, 1536:2048])
            nc.gpsimd.dma_start(
                J4[0:16, :].rearrange("r (q f) -> r q f", q=8),
                SW1[:, 256:512])
            nc.gpsimd.dma_start(
                I4[0:16, :].rearrange("r (q f) -> r q f", q=8),
                SW1[:, 512:640])
            nc.gpsimd.dma_start(
                J4[16:17, :].rearrange("r (q f) -> r q f", q=128),
                SW1[:, 656:672])
            nc.gpsimd.dma_start(
                I4[16:17, :].rearrange("r (q f) -> r q f", q=128),
                SW1[:, 672:680])
            nc.gpsimd.dma_start(
                Ju[:, :].bitcast(f16).rearrange("r (q f) -> r q f", q=64),
                SW1[:, 730:746])
            nc.gpsimd.dma_start(
                Iu[:, :].bitcast(f16).rearrange("r (q f) -> r q f", q=64),
                SW1[:, 746:754])
            nc.gpsimd.dma_start(
                J3[:, :].rearrange("r (q f) -> r q f", q=32),
                SW1[:, 754:802])
            nc.gpsimd.dma_start(
                I3[:, :].rearrange("r (q f) -> r q f", q=32),
                SW1[:, 802:826])
            chunk8(4)
            chunk8(5)
            # window lhsT per slot: rows 0..17 = [ones; Ci], 17..97 geometry (repack)
            Lw = []
            for s in range(W):
                Lt = cpool.tile([97, 2048], f16, tag=f"Lw{s}")
                nc.gpsimd.dma_start(
                    Lt[0:16, :].rearrange("r (q f) -> r q f", q=8),
                    SW2B[:, W * 256:W * 256 + 256])
                nc.gpsimd.dma_start(
                    Lt[16:17, :].rearrange("r (q f) -> r q f", q=128),
                    SW1[:, 714:730])
                Lw.append(Lt)
            chunk8(6)
            chunk8(7)
            nc.gpsimd.dma_start(
                PB[17:81, 0:GW].rearrange("r (q f) -> r q f", q=2),
                SW2A[:, :])
            nc.gpsimd.dma_start(
                PB[81:97, 0:GW].rearrange("r (q f) -> r q f", q=8),
                SW2B[:, 0:W * 256])
            chunk16(0)
            chunk16(1)

            psO = ctx.enter_context(tc.tile_pool(name="psO", bufs=4, space="PSUM"))
            wk = ctx.enter_context(tc.tile_pool(name="wk", bufs=2))
            wkr = ctx.enter_context(tc.tile_pool(name="wkr", bufs=4))
            stg = ctx.enter_context(tc.tile_pool(name="stg", bufs=5))
            dstv = outp.rearrange("(t p) nh -> p t nh", t=2)

            def emit_step(k, slot):
                """One output step: 8 (dense) or 16 (window) matmuls ->
                4 PSUM->SBUF fp16 copies -> one 1MB store."""
                dense = slot >= W
                # while stage-1's chain owns the DVE queue, ACT takes all
                # 4 PSUM->SBUF copies; otherwise split 2/2
                a = 3 if 2 <= k < 9 else 2
                qidx = 0
                st = stg.tile([128, 4096], f16, tag="st", name="st")
                for it in range(2):
                    for g in range(2):
                        po = psO.tile([128, 1024], f32, tag="po", name="po")
                        if dense:
                            for jl in range(2):
                                c0 = slot * 2048 + g * 1024 + jl * 512
                                nc.tensor.matmul(
                                    po[:, jl * 512:(jl + 1) * 512],
                                    LD8[0:18, :].rearrange(
                                        "k (two m) -> k two m",
                                        two=2)[:, :, it * 128:(it + 1) * 128],
                                    PB8[0:18, c0:c0 + 512].unsqueeze(1)
                                        .broadcast_to([18, 2, 512]),
                                    start=True, stop=True,
                                    perf_mode=mybir.MatmulPerfMode.DoubleRow)
                        else:
                            for jl in range(4):
                                js = g * 4 + jl
                                base = js * 256 + it * 128
                                nc.tensor.matmul(
                                    po[:, jl * 256:(jl + 1) * 256],
                                    Lw[slot][0:97, base:base + 128],
                                    PB[0:97, slot * 2048 + js * 256:slot * 2048 + (js + 1) * 256],
                                    start=True, stop=True)
                        dst = st[:, it * 2048 + g * 1024:it * 2048 + (g + 1) * 1024]
                        if qidx < a:
                            nc.scalar.copy(dst, po[:, :])
                        else:
                            nc.vector.tensor_copy(dst, po[:, :])
                        qidx += 1
                nc.sync.dma_start(
                    dstv[:, :, slot * 2048:(slot + 1) * 2048],
                    st[:, :].rearrange("p (t c) -> p t c", t=2))

            def emit_stage1_slot(s):
                """Geometry features for one window slot."""
                if True:
                    ps1 = psO.tile([128, 1024], f32, tag="po", name="ps1")
                    for b in range(B):
                        nc.tensor.matmul(ps1[:, b * 256:(b + 1) * 256],
                                         J4[:, (b * W + s) * 128:(b * W + s + 1) * 128],
                                         I4[:, b * IB:(b + 1) * IB],
                                         start=True, stop=True)
                    rc = wkr.tile([128, 1024], f32, tag="rc", name="rc")
                    nc.vector.reciprocal_approx_fast(rc[:, :], ps1[:, :])
                    ps2 = psO.tile([128, 1024], f32, tag="po", name="ps2")
                    nc.tensor.matmul(ps2[:, 0:256],
                                     Ju[:, s * 128:(s + 1) * 128], Iu[:, :],
                                     start=True, stop=True)
                    for k in range(3):
                        nc.tensor.matmul(ps2[:, 256 + k * 256:512 + k * 256],
                                         J3[:, (k * W + s) * 128:(k * W + s + 1) * 128],
                                         I3[:, k * IB:(k + 1) * IB],
                                         start=True, stop=True)
                    # drain ps2 to SBUF immediately (ACT) so the PSUM bank
                    # frees without waiting for the whole DVE chain; the PE
                    # can then roll straight into the next output step
                    pv = wkr.tile([128, 1024], f32, tag="pv", name="pv")
                    nc.scalar.copy(pv[:, :], ps2[:, :])
                    r2h = wk.tile([128, 512], f32, tag="r2h", name="r2h")
                    Rt = wk.tile([128, 256], f32, tag="Rt", name="Rt")
                    # gpsimd stays DMA-only (tensor ops there force a 14us
                    # ucode LOAD_LIB swap mid-kernel)
                    nc.vector.tensor_add(r2h[:, :], rc[:, 0:512], rc[:, 512:1024])
                    nc.vector.tensor_add(Rt[:, :], r2h[:, 0:256], r2h[:, 256:512])

                    F5 = wk.tile([128, 1280], f16, tag="F5", name="F5")
                    vt = wk.tile([128, 256], f32, tag="vt", name="vt")
                    nc.vector.tensor_scalar(vt[:, :], pv[:, 0:256], 0.0, None,
                                            op0=mybir.AluOpType.is_equal)
                    for k in range(3):
                        nc.vector.tensor_mul(F5[:, k * 256:(k + 1) * 256],
                                             vt[:, :], pv[:, 256 + k * 256:512 + k * 256])
                    nc.vector.tensor_mul(F5[:, 768:1024], vt[:, :], Rt[:, :])
                    nc.vector.tensor_copy(F5[:, 1024:1280], vt[:, :])

                    # repack: permuted partitions make each feature's src contiguous
                    for t in range(5):
                        dst = Lw[s][17 + t * 16:17 + (t + 1) * 16, :].rearrange(
                            "k (a i) -> k a i", a=8)
                        nc.gpsimd.dma_start(dst, F5[:, t * 256:(t + 1) * 256])

            # ---- emission order: dense steps lead (their operands arrive
            # earliest); stage-1 slots slot in one at a time between early
            # steps so the PE pipeline never fully drains; windows last ----
            dense_list = list(range(W, NJT))
            emit_step(0, dense_list[0])
            emit_step(1, dense_list[1])
            for s in range(W):
                emit_stage1_slot(s)
                emit_step(2 + s, dense_list[2 + s])
            post = dense_list[2 + W:9]
            for w in range(W):
                post.append(w)
                if 9 + w < len(dense_list):
                    post.append(dense_list[9 + w])
            for k, slot in enumerate(post):
                emit_step(2 + W + k, slot)
    nc.compile()
    _CACHE[key] = nc
    return nc


def make_in_maps(inputs):
    jf, itab, pc, C, cflat, m, uid = _host_tables(inputs)
    sigmas, W = _windows(m, uid)
    pcw16 = pc.astype(np.float16)             # [96, 2048]
    pcd16 = np.ascontiguousarray(pc[80:96]).astype(np.float16)  # [16, 2048] delta
    in_maps = []
    ru = 4 * KB
    # geometry rows tiled x W slots: [80, W*2048]
    geo = np.tile(pcw16[0:80], (1, W))
    wg1_all = np.ascontiguousarray(geo[0:64]).reshape(64, 2, W * 1024).reshape(128, W * 1024)
    wg2_all = np.ascontiguousarray(geo[64:80]).reshape(16, 8, W * 256).reshape(128, W * 256)

    def wideN(a):
        """[R, F] -> [128, R*F//128] staging layout (row r piece q -> part
        r*(128//R)+q), for R dividing 128."""
        r, f = a.shape
        q = 128 // r
        return np.ascontiguousarray(a).reshape(r, q, f // q).reshape(128, f // q)

    for c in range(NCORES):
        sl = slice(c * IB, (c + 1) * IB)
        sg = sigmas[c]
        jfw = np.concatenate([jf[:, t * 128:(t + 1) * 128] for t in sg[:W]], axis=1)
        cflP = np.concatenate([cflat[:, t * 2048:(t + 1) * 2048] for t in sg], axis=1)
        citc = np.ascontiguousarray(C.T[:, sl]).astype(np.float16)   # [16, IB]
        cf16 = np.ascontiguousarray(cflP).astype(np.float16)
        ldc = np.concatenate([np.ones((1, IB), np.float16), citc], 0)
        lwc = np.concatenate([np.ones((1, 2048), np.float16),
                              np.tile(citc, (1, 8))], 0)
        ifc = itab[:, sl]
        j4 = np.concatenate([jfw[b * KB:(b + 1) * KB] for b in range(B)],
                            axis=1).astype(np.float16)
        i4 = np.concatenate([ifc[b * KB:(b + 1) * KB] for b in range(B)],
                            axis=1).astype(np.float16)
        j3 = np.concatenate([jfw[ru + KU + KD * k:ru + KU + KD * (k + 1)]
                             for k in range(3)], axis=1).astype(np.float16)
        i3 = np.concatenate([ifc[ru + KU + KD * k:ru + KU + KD * (k + 1)]
                             for k in range(3)], axis=1).astype(np.float16)
        ju = np.ascontiguousarray(jfw[ru:ru + KU], np.float32)   # f32r bytes
        iu = np.ascontiguousarray(ifc[ru:ru + KU], np.float32)
        onehot = np.zeros((128, 16), np.float16)
        onehot[np.arange(128), np.arange(128) // 8] = 1.0
        # fp8 dense operands: Cj/Ci hi+lo planes (DoubleRow pairs)
        cjh, cjl = _fp8_hi_lo(cflP)                     # [1, 32768] each
        cjhl = np.concatenate([cjh, cjl], 0)            # [2, 32768] fp8
        cif = np.ascontiguousarray(C.T[:, sl], np.float32)  # [16, IB]
        cih, cil = _fp8_hi_lo(cif)
        ld8 = np.zeros((18, 2, IB), FP8)
        ld8[0, 0] = FP8(1.0)
        ld8[1, 0] = FP8(1.0)
        ld8[2:, 0] = cih
        ld8[2:, 1] = cil
        ld8 = ld8.reshape(18, 2 * IB)
        # one wide staging tensor holds every operand; segment offsets must
        # match the device-side redistributes in _build_program
        sw1 = np.zeros((128, 2048), np.float16)
        sw1[:, 0:256] = cf16.reshape(128, 256)
        sw1[:, 256:512] = wideN(j4[0:16])
        sw1[:, 512:640] = wideN(i4[0:16])
        sw1[:, 640:656] = onehot
        sw1[:, 656:672] = j4[16].reshape(128, 16)
        sw1[:, 672:680] = i4[16].reshape(128, 8)
        sw1[:, 680:712] = wideN(np.ascontiguousarray(ldc[0:16]))
        sw1[:, 712:714] = ldc[16].reshape(128, 2)
        sw1[:, 714:730] = lwc[16].reshape(128, 16)
        sw1[:, 730:746] = wideN(ju.view(np.float16))
        sw1[:, 746:754] = wideN(iu.view(np.float16))
        sw1[:, 754:802] = wideN(j3)
        sw1[:, 802:826] = wideN(i3)
        sw1[:, 832:1088] = wideN(cjhl).view(np.uint8).view(np.float16)
        sw1[:, 1088:1120] = wideN(np.ascontiguousarray(ld8[2:18])
                                  ).view(np.uint8).view(np.float16)
        sw1[:, 1120:1124] = wideN(np.ascontiguousarray(ld8[0:2])
                                  ).view(np.uint8).view(np.float16)
        sw1[:, 1124:1124 + W * 16] = np.tile(lwc[16:17], (1, W)).reshape(128, W * 16)
        sw2b = np.concatenate([wg2_all, wideN(np.ascontiguousarray(lwc[0:16])),
                               np.zeros((128, 2048 - W * 256 - 256), np.float16)],
                              axis=1)
        in_maps.append({
            "sw1": sw1,
            "sw2a": np.ascontiguousarray(wg1_all),
            "sw2b": np.ascontiguousarray(sw2b),
        })
    return in_maps, sigmas, W


def _assemble(res, sigmas):
    out = np.empty((1, N, N, H), np.float32)
    for c in range(NCORES):
        dev = np.asarray(res.results[c]["outp"]).astype(np.float32)
        dev = dev.reshape(IB, NJT, 128 * H)
        blk = out[0, c * IB:(c + 1) * IB].reshape(IB, NJT, 128 * H)
        blk[:, sigmas[c], :] = dev
    return out


def kernel(**inputs):
    from concourse import bass_utils
    in_maps, sigmas, W = make_in_maps(inputs)
    nc = _build_program(W)
    res = bass_utils.run_bass_kernel_spmd(nc, in_maps, core_ids=list(range(NCORES)))
    return _assemble(res, sigmas)


# revision 50
# speedup vs baseline: 1.1987x; 1.0085x over previous
"""Trainium2 Bass kernel for nn_CoordinateConditioning.

out[i,j,h] = v[i,j]*( (X[i]-X[j])@Wcoord[h] + Wdist[h]*R[i,j] + B*Wmask[h] )
             + C[i,h] + C[j,h]
with X = sum_b coords[b], R[i,j] = sum_b 1/(1+||x_b[i]-x_b[j]||^2),
v = pad/uid mask, C = B*c0 + gathered s_to_c sum.

Key structure exploited: ref_space_uid is sorted, so v[i,j] is a narrow
block-diagonal band.  For each core (256 i-rows) only W (~3) of the 16
j-tiles can contain v!=0 pairs; host computes the per-core window-tile
list from the actual inputs and the device program processes exactly
W "window" tiles (full geometry pipeline) + 16-W "dense" tiles
(out = C_i + C_j only, K=17 matmul with a shared stationary operand).
Per-core j-tile order is a host-chosen permutation sigma_c; the host
inverse-permutes the j-tile blocks when assembling the full output.

DMA layout: TRN2 SDMA engines only spray a transfer across all 16
engines when one side spans ~128 partitions; partition-narrow loads
(17/97 rows) land on ONE engine (~22 GB/s).  So the big constant
patterns are staged to SBUF as [128, f] HWDGE loads (sprayed), then
redistributed SBUF->SBUF with 128-partition sources via gpsimd SWDGE
(also sprayed).  Remaining tiny narrow loads ride the Act HWDGE ring so
their single-engine backlog cannot stall the SP ring that carries the
output stores.

Output is written fp16 (rel tolerance is 2e-2; fp16 rounding ~5e-4),
halving the dominant out-DMA traffic.  PSUM->SBUF copies are split
between ACT and DVE.
"""

import numpy as np
import ml_dtypes
from contextlib import ExitStack

FP8 = ml_dtypes.float8_e4m3fn


def _fp8_hi_lo(v):
    """fp8e4m3 hi/lo split: v ~ hi + lo with |err| <= |v| * 2^-8."""
    v = np.ascontiguousarray(v, dtype=np.float32)
    hi = v.astype(FP8)
    lo = (v - hi.astype(np.float32)).astype(FP8)
    return hi, lo

B, N, T, TOKEN_S, DIM_F, H = 4, 2048, 256, 384, 256, 16
NCORES = 8
IB = N // NCORES          # 256 i rows per core
NJT = N // 128            # 16 j tiles
KB = 17                   # K rows per batch for the r2 matmul
KU = 2                    # uid delta rows
KD = 4                    # rows per D_k
KF = 4 * KB + KU + 3 * KD # 82 total J/I feature rows
BIGM = 4096.0

_CACHE = {}


def _split_hi_lo(v):
    """fp16-exact hi/lo split (hi keeps 10 mantissa bits)."""
    v = np.ascontiguousarray(v, dtype=np.float32)
    hi = (v.view(np.uint32) & np.uint32(0xFFFFE000)).view(np.float32)
    return hi, (v - hi).astype(np.float32)


def _host_tables(inputs):
    I = {k: np.asarray(v) for k, v in inputs.items()}
    x = np.ascontiguousarray(I['atom_coords_noisy'], dtype=np.float32)  # [B,N,3]
    m = I['atom_pad_mask'].reshape(-1).astype(np.float32)               # [N]
    uid = I['ref_space_uid'].reshape(-1).astype(np.float32)             # [N]

    # ---- small linears (replicated) ----
    def ln(v, g, b, eps=1e-5):
        mu = v.mean(-1, keepdims=True)
        var = ((v - mu) ** 2).mean(-1, keepdims=True)
        return (v - mu) / np.sqrt(var + eps) * g + b

    s = np.concatenate([I['s_trunk'], I['s_inputs']], -1).astype(np.float32) @ I['W_single'].T
    fe = np.cos(2 * np.pi * (I['times'][:, None] * I['Wf'][:, 0][None, :] + I['bf'])).astype(np.float32)
    s = s + (ln(fe, I['ln_f_g'], I['ln_f_b']) @ I['Wf2s'].T)[:, None, :]
    s2c = ln(s, I['ln_s_g'], I['ln_s_b']) @ I['Wsc'].T                  # [B,T,1]
    ssum = s2c[:, :, 0].sum(0)                                          # [T]
    tok = I['atom_to_token_idx'].reshape(-1).astype(np.int64)
    S = ssum[tok]                                                       # [N]
    af = np.concatenate([I['ref_pos'][0], I['ref_charge'][0][:, None],
                         I['ref_element'][0]], -1).astype(np.float32)   # [N,132]
    c0 = af @ I['Wa'].T + I['ba']                                       # [N,16]
    C = (B * c0 + S[:, None]).astype(np.float32)                        # [N,16]

    X = x.sum(0)                                                        # [N,3]
    Wc = np.asarray(I['Wcoord'], np.float32)                            # [16,3]
    # device feature maps hold X_j - X_i, the formula needs X_i - X_j -> negate
    wtab = np.stack([-Wc[:, 0], -Wc[:, 1], -Wc[:, 2],
                     np.asarray(I['Wdist'], np.float32)[:, 0],
                     B * np.asarray(I['Wmask'], np.float32)[:, 0]], 0)  # [5,16]

    # ---- J/I feature tables for the per-(j,i) matmuls ----
    n2 = np.einsum('bnk,bnk->bn', x.astype(np.float64), x.astype(np.float64)).astype(np.float32)
    ones = np.ones(N, np.float32)
    jf = np.zeros((KF, N), np.float32)
    itab = np.zeros((KF, N), np.float32)
    for b in range(B):
        r = b * KB
        for k in range(3):
            xh, xl = _split_hi_lo(x[b, :, k])
            jf[r + 4 * k + 0] = xh
            jf[r + 4 * k + 1] = xh
            jf[r + 4 * k + 2] = xl
            jf[r + 4 * k + 3] = xl
            itab[r + 4 * k + 0] = -2.0 * xh
            itab[r + 4 * k + 1] = -2.0 * xl
            itab[r + 4 * k + 2] = -2.0 * xh
            itab[r + 4 * k + 3] = -2.0 * xl
        nh, nl = _split_hi_lo(n2[b])
        jf[r + 12], jf[r + 13] = nh, nl
        itab[r + 12], itab[r + 13] = ones, ones
        jf[r + 14], jf[r + 15] = ones, ones
        itab[r + 14], itab[r + 15] = nh, nl
        jf[r + 16] = ones
        itab[r + 16] = ones
    ru = 4 * KB
    jf[ru] = uid + BIGM * (1.0 - m)
    itab[ru] = ones
    jf[ru + 1] = ones
    itab[ru + 1] = -uid + BIGM * (1.0 - m)
    for k in range(3):
        r = ru + KU + KD * k
        Xh, Xl = _split_hi_lo(X[:, k])
        jf[r + 0], jf[r + 1] = Xh, Xl
        itab[r + 0], itab[r + 1] = ones, ones
        jf[r + 2], jf[r + 3] = ones, ones
        itab[r + 2], itab[r + 3] = -Xh, -Xl

    # ---- constant rhs pattern rows (per 256-col jsub block) ----
    # rows 0..79: geometry block-diag(delta_jp * wtab[t]); 80..95: delta_h
    blk = np.zeros((96, 256), np.float32)
    for t in range(5):
        for jp in range(16):
            blk[t * 16 + jp, jp * 16:(jp + 1) * 16] = wtab[t]
    for hp in range(16):
        blk[80 + hp, hp::16] = 1.0
    pc = np.tile(blk, (1, 8))                                           # [96, 2048]

    # bake the per-tile column permutation p -> j = (p%8)*16 + p//8 into jf
    # so device lhsT slices are plain contiguous (walrus: one free dim only)
    p = np.arange(128)
    perm = (np.arange(N) // 128) * 128 + ((p % 8) * 16 + p // 8)[np.tile(p, N // 128) * 0 + np.arange(N) % 128]
    jf = np.ascontiguousarray(jf[:, perm])

    cflat = C.reshape(1, N * H).astype(np.float32)
    return jf, itab, pc, C, cflat, m.astype(bool), uid


def _windows(m, uid):
    """Per-core window j-tile lists (tiles that can hold v!=0 pairs) and
    the per-core slot->j-tile permutation sigma (window tiles first)."""
    tiles_per_core = []
    for c in range(NCORES):
        sl = slice(c * IB, (c + 1) * IB)
        vi = m[sl]
        if vi.any():
            U = np.unique(uid[sl][vi])
            pj = np.where(m & np.isin(uid, U))[0]
            tiles = sorted(set((pj // 128).tolist()))
        else:
            tiles = []
        tiles_per_core.append(tiles)
    W = max(1, max(len(t) for t in tiles_per_core))
    sigmas = []
    for tiles in tiles_per_core:
        rest = [t for t in range(NJT) if t not in tiles]
        pad = rest[:W - len(tiles)]
        rest2 = rest[W - len(tiles):]
        sigmas.append(np.array(tiles + pad + rest2, np.int64))
    return sigmas, W


def _build_program(W):
    key = ('nc', W)
    if key in _CACHE:
        return _CACHE[key]
    import concourse.bass as bass
    import concourse.bacc as bacc
    import concourse.tile as tile
    from concourse import mybir

    f32 = mybir.dt.float32
    f32r = mybir.dt.float32r
    f16 = mybir.dt.float16
    f8 = mybir.dt.float8e4

    GW = W * 2048            # valid geometry columns in the merged tile

    nc = bacc.Bacc("TRN2", target_bir_lowering=False, debug=False)
    # wide staging payloads (all sprayed as [128, f] loads, then SWDGE-
    # redistributed with 128-partition sources into the narrow operand
    # layouts).  sw1 feeds the dense steps + stage 1, sw2 the window steps.
    # The delta_h pattern is generated on device (17 strided memsets), not
    # loaded: it would otherwise dominate the dense-path critical DMA.
    #   sw1 = [ Cjflat(256) | J4 rows0-15(256) | I4 rows0-15(128) | delta16(16) | pad ]
    #   sw2a = [ geo rows0-63 (W*1024) ]   (8KB rows, the proven-fast shape)
    #   sw2b = [ geo rows64-79 (W*256) | lwc rows0-15(256) | pad ]
    FW1 = 2048
    # dense-critical payload in its own small tensor so the first dense
    # steps do not wait for the whole SW1 load:
    #   sw0 = [ CjHL fp8 (256) | LD8 rows2-17 (32) | LD8 rows0-1 (4) |
    #           onehot (16) | pad ]
    sw0 = nc.dram_tensor("sw0", [128, 512], f16, kind="ExternalInput").ap()
    sw1 = nc.dram_tensor("sw1", [128, FW1], f16, kind="ExternalInput").ap()
    sw2a = nc.dram_tensor("sw2a", [128, W * 1024], f16, kind="ExternalInput").ap()
    FW2B = 2048
    sw2b = nc.dram_tensor("sw2b", [128, FW2B], f16, kind="ExternalInput").ap()
    outp = nc.dram_tensor("outp", [IB, N * H], f16, kind="ExternalOutput").ap()

    with tile.TileContext(nc) as tc:
        with ExitStack() as ctx:
            cpool = ctx.enter_context(tc.tile_pool(name="const", bufs=1))
            J4 = cpool.tile([KB, B * W * 128], f16, tag="J4")
            I4 = cpool.tile([KB, B * IB], f16, tag="I4")
            J3 = cpool.tile([KD, 3 * W * 128], f16, tag="J3")
            I3 = cpool.tile([KD, 3 * IB], f16, tag="I3")
            Ju = cpool.tile([KU, W * 128], f32r, tag="Ju")
            Iu = cpool.tile([KU, IB], f32r, tag="Iu")
            # window rhs pattern tile (fp16): rows 0..16 = [Cj; delta_h],
            # rows 17..96 = geometry; only window columns (< GW) are used
            PB = cpool.tile([97, GW], f16, tag="PB")
            # dense rhs pattern tile (fp8e4, DoubleRow): row 0 = Cj_hi,
            # row 1 = Cj_lo, rows 2..17 = delta_h; the DoubleRow pair dim is
            # stride-0 on this operand
            PB8 = cpool.tile([18, NJT * 2048], f8, tag="PB8")
            # dense lhsT fp8 pairs: row 0/1 = (1,0), rows 2..17 = (CiH, CiL)
            LD8 = cpool.tile([18, 2 * IB], f8, tag="LD8")
            # window lhsT helper: row 0 = ones, rows 1..17 = Ci^T
            Ld = cpool.tile([17, IB], f16, tag="Ld")
            # staging tiles for the wide loads (sprayed across all 16 SDMA
            # engines), redistributed below with 128-partition sources
            SW0 = cpool.tile([128, 512], f16, tag="SW0")
            SW1 = cpool.tile([128, FW1], f16, tag="SW1")
            SW2A = cpool.tile([128, W * 1024], f16, tag="SW2A")
            SW2B = cpool.tile([128, FW2B], f16, tag="SW2B")
            SD = cpool.tile([128, 1024], f16, tag="SD")
            SD8 = cpool.tile([128, 4096], f8, tag="SD8")

            # wide staged loads on the SP HWDGE ring (shared with stores)
            nc.sync.dma_start(SW0[:, :], sw0[:, :])
            nc.sync.dma_start(SW1[:, :], sw1[:, :])
            nc.sync.dma_start(SW2A[:, :], sw2a[:, :])
            nc.sync.dma_start(SW2B[:, :], sw2b[:, :])

            # delta_h staging, generated on device: partition p holds the
            # 16-periodic one-hot(p//8) pattern, expanded by DVE broadcast
            # (fp16 copy for the window tile, fp8 cast for the dense tile)
            nc.vector.tensor_copy(
                SD8[:, :].rearrange("p (a s) -> p a s", s=16),
                SW0[:, 292:308].unsqueeze(1).broadcast_to([128, 256, 16]))
            nc.vector.tensor_copy(
                SD[:, :].rearrange("p (a s) -> p a s", s=16),
                SW0[:, 292:308].unsqueeze(1).broadcast_to([128, 64, 16]))

            # Everything is SWDGE-redistributed out of the wide staging
            # tiles; there are no partition-narrow DRAM loads at all (they
            # hot-spot one SDMA engine and gate every completion).  SWDGE
            # assigns each dma_start to ONE SDMA engine (~25 GB/s), so the
            # big patterns move as ~128KB pieces across several
            # instructions.  Order tracks consumption: dense-path operands
            # first, stage-1 operands, then the window-path operands.
            def chunk16(c):
                nc.gpsimd.dma_start(
                    PB[0:1, c * 4096:(c + 1) * 4096].rearrange(
                        "r (q f) -> r q f", q=16),
                    SW1[c * 16:(c + 1) * 16, 0:256])
                nc.gpsimd.dma_start(
                    PB[1:17, c * 4096:(c + 1) * 4096].rearrange(
                        "r (q f) -> r q f", q=8),
                    SD[:, c * 512:(c + 1) * 512])

            def chunk8(c):
                nc.gpsimd.dma_start(
                    PB8[2:18, c * 4096:(c + 1) * 4096].rearrange(
                        "r (q f) -> r q f", q=8),
                    SD8[:, c * 512:(c + 1) * 512])

            nc.gpsimd.dma_start(
                LD8[2:18, :].rearrange("r (q f) -> r q f", q=8),
                SW0[:, 256:288].bitcast(f8))
            nc.gpsimd.dma_start(
                LD8[0:2, :].rearrange("r (q f) -> r q f", q=64),
                SW0[:, 288:292].bitcast(f8))
            nc.gpsimd.dma_start(
                PB8[0:2, :].rearrange("r (q f) -> r q f", q=64),
                SW0[:, 0:256].bitcast(f8))
            # first dense steps gate on these; small pieces complete in
            # ~1.5-3us each on their SWDGE engines
            nc.gpsimd.dma_start(
                PB8[2:18, 8192:10240].rearrange("r (q f) -> r q f", q=8),
                SD8[:, 1024:1280])
            nc.gpsimd.dma_start(
                PB8[2:18, 10240:12288].rearrange("r (q f) -> r q f", q=8),
                SD8[:, 1280:1536])
            nc.gpsimd.dma_start(
                PB8[2:18, 12288:16384].rearrange("r (q f) -> r q f", q=8),
                SD8[:, 1536:2048])
            nc.gpsimd.dma_start(
                J4[0:16, :].rearrange("r (q f) -> r q f", q=8),
                SW1[:, 256:512])
            nc.gpsimd.dma_start(
                I4[0:16, :].rearrange("r (q f) -> r q f", q=8),
                SW1[:, 512:640])
            nc.gpsimd.dma_start(
                J4[16:17, :].rearrange("r (q f) -> r q f", q=128),
                SW1[:, 656:672])
            nc.gpsimd.dma_start(
                I4[16:17, :].rearrange("r (q f) -> r q f", q=128),
                SW1[:, 672:680])
            nc.gpsimd.dma_start(
                Ju[:, :].bitcast(f16).rearrange("r (q f) -> r q f", q=64),
                SW1[:, 730:746])
            nc.gpsimd.dma_start(
                Iu[:, :].bitcast(f16).rearrange("r (q f) -> r q f", q=64),
                SW1[:, 746:754])
            nc.gpsimd.dma_start(
                J3[:, :].rearrange("r (q f) -> r q f", q=32),
                SW1[:, 754:802])
            nc.gpsimd.dma_start(
                I3[:, :].rearrange("r (q f) -> r q f", q=32),
                SW1[:, 802:826])
            chunk8(4)
            chunk8(5)
            # window lhsT per slot: rows 0..17 = [ones; Ci], 17..97 geometry (repack)
            Lw = []
            for s in range(W):
                Lt = cpool.tile([97, 2048], f16, tag=f"Lw{s}")
                nc.gpsimd.dma_start(
                    Lt[0:16, :].rearrange("r (q f) -> r q f", q=8),
                    SW2B[:, W * 256:W * 256 + 256])
                nc.gpsimd.dma_start(
                    Lt[16:17, :].rearrange("r (q f) -> r q f", q=128),
                    SW1[:, 714:730])
                Lw.append(Lt)
            chunk8(6)
            chunk8(7)
            nc.gpsimd.dma_start(
                PB[17:81, 0:GW].rearrange("r (q f) -> r q f", q=2),
                SW2A[:, :])
            nc.gpsimd.dma_start(
                PB[81:97, 0:GW].rearrange("r (q f) -> r q f", q=8),
                SW2B[:, 0:W * 256])
            chunk16(0)
            chunk16(1)

            psO = ctx.enter_context(tc.tile_pool(name="psO", bufs=4, space="PSUM"))
            wk = ctx.enter_context(tc.tile_pool(name="wk", bufs=2))
            wkr = ctx.enter_context(tc.tile_pool(name="wkr", bufs=4))
            stg = ctx.enter_context(tc.tile_pool(name="stg", bufs=5))
            dstv = outp.rearrange("(t p) nh -> p t nh", t=2)

            def emit_step(k, slot):
                """One output step: 8 (dense) or 16 (window) matmuls ->
                4 PSUM->SBUF fp16 copies -> one 1MB store."""
                dense = slot >= W
                # while stage-1's chain owns the DVE queue, ACT takes all
                # 4 PSUM->SBUF copies; otherwise split 2/2
                a = 3 if 2 <= k < 9 else 2
                qidx = 0
                st = stg.tile([128, 4096], f16, tag="st", name="st")
                for it in range(2):
                    for g in range(2):
                        po = psO.tile([128, 1024], f32, tag="po", name="po")
                        if dense:
                            for jl in range(2):
                                c0 = slot * 2048 + g * 1024 + jl * 512
                                nc.tensor.matmul(
                                    po[:, jl * 512:(jl + 1) * 512],
                                    LD8[0:18, :].rearrange(
                                        "k (two m) -> k two m",
                                        two=2)[:, :, it * 128:(it + 1) * 128],
                                    PB8[0:18, c0:c0 + 512].unsqueeze(1)
                                        .broadcast_to([18, 2, 512]),
                                    start=True, stop=True,
                                    perf_mode=mybir.MatmulPerfMode.DoubleRow)
                        else:
                            for jl in range(4):
                                js = g * 4 + jl
                                base = js * 256 + it * 128
                                nc.tensor.matmul(
                                    po[:, jl * 256:(jl + 1) * 256],
                                    Lw[slot][0:97, base:base + 128],
                                    PB[0:97, slot * 2048 + js * 256:slot * 2048 + (js + 1) * 256],
                                    start=True, stop=True)
                        dst = st[:, it * 2048 + g * 1024:it * 2048 + (g + 1) * 1024]
                        if qidx < a:
                            nc.scalar.copy(dst, po[:, :])
                        else:
                            nc.vector.tensor_copy(dst, po[:, :])
                        qidx += 1
                nc.sync.dma_start(
                    dstv[:, :, slot * 2048:(slot + 1) * 2048],
                    st[:, :].rearrange("p (t c) -> p t c", t=2))

            def emit_stage1_slot(s):
                """Geometry features for one window slot."""
                if True:
                    ps1 = psO.tile([128, 1024], f32, tag="po", name="ps1")
                    for b in range(B):
                        nc.tensor.matmul(ps1[:, b * 256:(b + 1) * 256],
                                         J4[:, (b * W + s) * 128:(b * W + s + 1) * 128],
                                         I4[:, b * IB:(b + 1) * IB],
                                         start=True, stop=True)
                    rc = wkr.tile([128, 1024], f32, tag="rc", name="rc")
                    nc.vector.reciprocal_approx_fast(rc[:, :], ps1[:, :])
                    ps2 = psO.tile([128, 1024], f32, tag="po", name="ps2")
                    nc.tensor.matmul(ps2[:, 0:256],
                                     Ju[:, s * 128:(s + 1) * 128], Iu[:, :],
                                     start=True, stop=True)
                    for k in range(3):
                        nc.tensor.matmul(ps2[:, 256 + k * 256:512 + k * 256],
                                         J3[:, (k * W + s) * 128:(k * W + s + 1) * 128],
                                         I3[:, k * IB:(k + 1) * IB],
                                         start=True, stop=True)
                    # drain ps2 to SBUF immediately (ACT) so the PSUM bank
                    # frees without waiting for the whole DVE chain; the PE
                    # can then roll straight into the next output step
                    pv = wkr.tile([128, 1024], f32, tag="pv", name="pv")
                    nc.scalar.copy(pv[:, :], ps2[:, :])
                    r2h = wk.tile([128, 512], f32, tag="r2h", name="r2h")
                    Rt = wk.tile([128, 256], f32, tag="Rt", name="Rt")
                    # gpsimd stays DMA-only (tensor ops there force a 14us
                    # ucode LOAD_LIB swap mid-kernel)
                    nc.vector.tensor_add(r2h[:, :], rc[:, 0:512], rc[:, 512:1024])
                    nc.vector.tensor_add(Rt[:, :], r2h[:, 0:256], r2h[:, 256:512])

                    F5 = wk.tile([128, 1280], f16, tag="F5", name="F5")
                    vt = wk.tile([128, 256], f32, tag="vt", name="vt")
                    nc.vector.tensor_scalar(vt[:, :], pv[:, 0:256], 0.0, None,
                                            op0=mybir.AluOpType.is_equal)
                    for k in range(3):
                        nc.vector.tensor_mul(F5[:, k * 256:(k + 1) * 256],
                                             vt[:, :], pv[:, 256 + k * 256:512 + k * 256])
                    nc.vector.tensor_mul(F5[:, 768:1024], vt[:, :], Rt[:, :])
                    nc.vector.tensor_copy(F5[:, 1024:1280], vt[:, :])

                    # repack: permuted partitions make each feature's src contiguous
                    for t in range(5):
                        dst = Lw[s][17 + t * 16:17 + (t + 1) * 16, :].rearrange(
                            "k (a i) -> k a i", a=8)
                        nc.gpsimd.dma_start(dst, F5[:, t * 256:(t + 1) * 256])

            # ---- emission order: dense steps lead (their operands arrive
            # earliest); stage-1 slots slot in one at a time between early
            # steps so the PE pipeline never fully drains; windows last ----
            dense_list = list(range(W, NJT))
            emit_step(0, dense_list[0])
            emit_step(1, dense_list[1])
            for s in range(W):
                emit_stage1_slot(s)
                emit_step(2 + s, dense_list[2 + s])
            post = dense_list[2 + W:9]
            for w in range(W):
                post.append(w)
                if 9 + w < len(dense_list):
                    post.append(dense_list[9 + w])
            for k, slot in enumerate(post):
                emit_step(2 + W + k, slot)
    nc.compile()
    _CACHE[key] = nc
    return nc


def make_in_maps(inputs):
    jf, itab, pc, C, cflat, m, uid = _host_tables(inputs)
    sigmas, W = _windows(m, uid)
    pcw16 = pc.astype(np.float16)             # [96, 2048]
    pcd16 = np.ascontiguousarray(pc[80:96]).astype(np.float16)  # [16, 2048] delta
    in_maps = []
    ru = 4 * KB
    # geometry rows tiled x W slots: [80, W*2048]
    geo = np.tile(pcw16[0:80], (1, W))
    wg1_all = np.ascontiguousarray(geo[0:64]).reshape(64, 2, W * 1024).reshape(128, W * 1024)
    wg2_all = np.ascontiguousarray(geo[64:80]).reshape(16, 8, W * 256).reshape(128, W * 256)

    def wideN(a):
        """[R, F] -> [128, R*F//128] staging layout (row r piece q -> part
        r*(128//R)+q), for R dividing 128."""
        r, f = a.shape
        q = 128 // r
        return np.ascontiguousarray(a).reshape(r, q, f // q).reshape(128, f // q)

    for c in range(NCORES):
        sl = slice(c * IB, (c + 1) * IB)
        sg = sigmas[c]
        jfw = np.concatenate([jf[:, t * 128:(t + 1) * 128] for t in sg[:W]], axis=1)
        cflP = np.concatenate([cflat[:, t * 2048:(t + 1) * 2048] for t in sg], axis=1)
        citc = np.ascontiguousarray(C.T[:, sl]).astype(np.float16)   # [16, IB]
        cf16 = np.ascontiguousarray(cflP).astype(np.float16)
        ldc = np.concatenate([np.ones((1, IB), np.float16), citc], 0)
        lwc = np.concatenate([np.ones((1, 2048), np.float16),
                              np.tile(citc, (1, 8))], 0)
        ifc = itab[:, sl]
        j4 = np.concatenate([jfw[b * KB:(b + 1) * KB] for b in range(B)],
                            axis=1).astype(np.float16)
        i4 = np.concatenate([ifc[b * KB:(b + 1) * KB] for b in range(B)],
                            axis=1).astype(np.float16)
        j3 = np.concatenate([jfw[ru + KU + KD * k:ru + KU + KD * (k + 1)]
                             for k in range(3)], axis=1).astype(np.float16)
        i3 = np.concatenate([ifc[ru + KU + KD * k:ru + KU + KD * (k + 1)]
                             for k in range(3)], axis=1).astype(np.float16)
        ju = np.ascontiguousarray(jfw[ru:ru + KU], np.float32)   # f32r bytes
        iu = np.ascontiguousarray(ifc[ru:ru + KU], np.float32)
        onehot = np.zeros((128, 16), np.float16)
        onehot[np.arange(128), np.arange(128) // 8] = 1.0
        # fp8 dense operands: Cj/Ci hi+lo planes (DoubleRow pairs)
        cjh, cjl = _fp8_hi_lo(cflP)                     # [1, 32768] each
        cjhl = np.concatenate([cjh, cjl], 0)            # [2, 32768] fp8
        cif = np.ascontiguousarray(C.T[:, sl], np.float32)  # [16, IB]
        cih, cil = _fp8_hi_lo(cif)
        ld8 = np.zeros((18, 2, IB), FP8)
        ld8[0, 0] = FP8(1.0)
        ld8[1, 0] = FP8(1.0)
        ld8[2:, 0] = cih
        ld8[2:, 1] = cil
        ld8 = ld8.reshape(18, 2 * IB)
        # one wide staging tensor holds every operand; segment offsets must
        # match the device-side redistributes in _build_program
        sw1 = np.zeros((128, 2048), np.float16)
        sw1[:, 0:256] = cf16.reshape(128, 256)
        sw1[:, 256:512] = wideN(j4[0:16])
        sw1[:, 512:640] = wideN(i4[0:16])
        sw1[:, 640:656] = onehot
        sw1[:, 656:672] = j4[16].reshape(128, 16)
        sw1[:, 672:680] = i4[16].reshape(128, 8)
        sw1[:, 680:712] = wideN(np.ascontiguousarray(ldc[0:16]))
        sw1[:, 712:714] = ldc[16].reshape(128, 2)
        sw1[:, 714:730] = lwc[16].reshape(128, 16)
        sw1[:, 730:746] = wideN(ju.view(np.float16))
        sw1[:, 746:754] = wideN(iu.view(np.float16))
        sw1[:, 754:802] = wideN(j3)
        sw1[:, 802:826] = wideN(i3)
        sw0 = np.zeros((128, 512), np.float16)
        sw0[:, 0:256] = wideN(cjhl).view(np.uint8).view(np.float16)
        sw0[:, 256:288] = wideN(np.ascontiguousarray(ld8[2:18])
                                ).view(np.uint8).view(np.float16)
        sw0[:, 288:292] = wideN(np.ascontiguousarray(ld8[0:2])
                                ).view(np.uint8).view(np.float16)
        sw0[:, 292:308] = onehot
        sw1[:, 1124:1124 + W * 16] = np.tile(lwc[16:17], (1, W)).reshape(128, W * 16)
        sw2b = np.concatenate([wg2_all, wideN(np.ascontiguousarray(lwc[0:16])),
                               np.zeros((128, 2048 - W * 256 - 256), np.float16)],
                              axis=1)
        in_maps.append({
            "sw0": sw0,
            "sw1": sw1,
            "sw2a": np.ascontiguousarray(wg1_all),
            "sw2b": np.ascontiguousarray(sw2b),
        })
    return in_maps, sigmas, W


def _assemble(res, sigmas):
    out = np.empty((1, N, N, H), np.float32)
    for c in range(NCORES):
        dev = np.asarray(res.results[c]["outp"]).astype(np.float32)
        dev = dev.reshape(IB, NJT, 128 * H)
        blk = out[0, c * IB:(c + 1) * IB].reshape(IB, NJT, 128 * H)
        blk[:, sigmas[c], :] = dev
    return out


def kernel(**inputs):
    from concourse import bass_utils
    in_maps, sigmas, W = make_in_maps(inputs)
    nc = _build_program(W)
    res = bass_utils.run_bass_kernel_spmd(nc, in_maps, core_ids=list(range(NCORES)))
    return _assemble(res, sigmas)
